# revision 5
# baseline (speedup 1.0000x reference)
# Trainium2 Bass kernel for CrossAttentionFusion — fp8 DoubleRow + transposed
# PV rewrite.
#
# Reference computation (per batch b):
#   pet_seq = pet_feat[b] viewed as (C, L)^T            L = H*W = 4096, C = 512
#   q = pet_seq @ Wq.T ; k = ct_seq @ Wk.T ; v = ct_seq @ Wv.T   (8 heads, hd=64)
#   x = softmax(q k^T / sqrt(hd)) v                      per head
#   y = LN(pet_seq + x @ Wp.T + bp) * gamma + beta       -> (B, C, H, W)
#
# Sharding: 8 cores = 2 batches x 4 query-row chunks (1024 rows each), no
# collectives.
#
# Design notes (v2 — tuned against the TimelineSim cost model):
#   * The kernel is elementwise-bound: the softmax exp must read f32 PSUM and
#     write fp8 SBUF, which only ACT/DVE can do (GPSIMD has no PSUM port, DMA
#     cannot touch PSUM).  All structural choices minimize non-exp ACT/DVE
#     work and keep both engines saturated.
#   * Scores: fp8e4 DoubleRow with a stride-0 broadcast contraction (doubles
#     the product; q side pre-scaled by A8*scale/2 so PSUM = A8*logit).
#   * exp runs split across ACT (table exp) and DVE (Schraudolph int8 bit
#     trick) chosen per-op by a cost-model greedy balancer.
#   * PV runs TRANSPOSED: P8 is the matmul stationary ([128k, 2, 128q]
#     slices), V the moving operand, so O^T lands as [128 q-part, 66] with
#     the softmax denominator (ones column in V) at column 64.  The
#     normalization is then ONE tensor_tensor divide per head with a
#     stride-0 broadcast of the per-partition denominator — no reciprocal
#     chain, no PE broadcasts, no denominator DMAs.
#   * xt is produced in bf16 [q, hd] layout and flipped back to [hd, q] with
#     32 XBAR DMA transposes (14 ns/16x128 tile, runs on the idle DMA path).
#     Out-projection is a plain bf16 matmul (PE has large slack).
#   * LayerNorm: channel sums via ones-column f32r matmuls, row stats on
#     [1,512] strips, apply on GPSIMD (SBUF-only) with gamma/beta as
#     per-partition tensor_scalar.
#   * All projection/copy/post work is queued as micro-tasks popped inside
#     the attention dm-loop so ACT/DVE/PE never drain between phases.

import numpy as np
import ml_dtypes

import concourse.bacc as bacc
import concourse.bass as bass
import concourse.tile as tile
from concourse import mybir
from concourse import bass_utils
from concourse.alu_op_type import AluOpType
from contextlib import ExitStack

F32 = mybir.dt.float32
F32R = mybir.dt.float32r
BF16 = mybir.dt.bfloat16
FP8 = mybir.dt.float8e4
I8 = mybir.dt.int8
E4 = ml_dtypes.float8_e4m3
DR = mybir.MatmulPerfMode.DoubleRow

B, C, H, W = 2, 512, 64, 64
L = H * W                    # 4096
NH, HD = 8, 64
NCORES = 8
LQ = L // 4                  # 1024 query rows per core
ATT_SCALE = HD ** -0.5       # 1/8
LN_EPS = 1e-5

# fp8 Schraudolph constants (e4m3: 4 exp bits bias 7, 3 mantissa bits).
A8 = 8.0 / np.log(2.0)            # octave slope in bit units
B8 = 56.05                        # 7*8 + truncation/Schraudolph tuning
B8EFF = 32.0                      # bias added before int8 trunc
SH = (B8 - B8EFF) / A8            # effective logit shift (~2.084)
QPRESCALE = float(A8 * ATT_SCALE / 2.0)   # pet8 host prescale (DR doubles)
SC_ACT = float(1.0 / A8)
BIAS_ACT = float(-SH)

NC4 = C // 128           # 4 channel chunks of 128
NM = L // 128            # 32 key m-chunks
NDM = NM // 2            # 16 double m-chunks
NLQ = LQ // 512          # 2 query chunks per core
NQB = 4                  # 128-query blocks per lc chunk


def dr0(ap):
    """Stride-0 DoubleRow plane dim: [K, N] -> [K, 2, N] reading data twice."""
    return ap.unsqueeze(1).broadcast_to((ap.shape[0], 2, ap.shape[1]))


class EngineBalancer:
    """Greedy engine assignment using the v2 instruction-cost model.

    engine-busy ns = slope*cols + init/2 where init = 2*access_cycles of the
    slowest operand space.  GPSIMD adds the Q7 launch and runs TT at 0.42
    efficiency.  GPSIMD cannot access PSUM.
    """
    SLOPE = {"act": 0.8333, "dve": 1.0417, "gps": 1.3889}
    INIT_PSUM = {"act": 143.0, "dve": 125.0}
    INIT_SBUF = {"act": 185.0, "dve": 60.0, "gps": 120.0}

    def __init__(self):
        self.busy = {"act": 0.0, "dve": 0.0, "gps": 0.0}

    def cost(self, e, cols, psum=True, tt=False):
        c = self.SLOPE[e] * cols
        if e == "gps":
            if tt:
                c = cols * 0.8333 / 0.42
            return c + self.INIT_SBUF["gps"]
        init = self.INIT_PSUM[e] if psum else self.INIT_SBUF[e]
        return c + init

    def pick(self, cols, force=None, candidates=("act", "dve"), psum=True,
             tt=False):
        if force is None:
            e = min(candidates,
                    key=lambda k: self.busy[k] + self.cost(k, cols, psum, tt))
        else:
            e = force
        self.busy[e] += self.cost(e, cols, psum, tt)
        return e


def build_nc(debug: bool = False, debug_taps: bool = False):
    nc = bacc.Bacc("TRN2", target_bir_lowering=False, debug=debug,
                   num_devices=NCORES)
    eng = {"act": nc.scalar, "dve": nc.vector, "gps": nc.gpsimd}
    bal = EngineBalancer()

    def e_copy(dst, src, cols, force=None, psum=True):
        which = bal.pick(cols, force, psum=psum)
        if which == "act":
            nc.scalar.copy(dst, src)
        else:
            eng[which].tensor_copy(dst, src)

    # ---- DRAM I/O ----------------------------------------------------------
    pet8_d = nc.dram_tensor("pet8", [2, 128, 2, LQ], FP8, kind="ExternalInput").ap()
    ct8_d = nc.dram_tensor("ct8", [2, 128, 2, L], FP8, kind="ExternalInput").ap()
    pet16_d = nc.dram_tensor("pet16", [C, LQ], BF16, kind="ExternalInput").ap()
    wq8_d = nc.dram_tensor("wq8", [2, 128, 2, C], FP8, kind="ExternalInput").ap()
    wk8_d = nc.dram_tensor("wk8", [2, 128, 2, C], FP8, kind="ExternalInput").ap()
    wv8_d = nc.dram_tensor("wv8", [2, 128, 2, C], FP8, kind="ExternalInput").ap()
    wp16_d = nc.dram_tensor("wp16", [NC4, 128, C], BF16, kind="ExternalInput").ap()
    gamma_d = nc.dram_tensor("gamma", [C, 1], F32, kind="ExternalInput").ap()
    beta_d = nc.dram_tensor("beta", [C, 1], F32, kind="ExternalInput").ap()
    bp_d = nc.dram_tensor("bp", [C, 1], F32, kind="ExternalInput").ap()
    out_d = nc.dram_tensor("out", [C, LQ], F32, kind="ExternalOutput").ap()
    taps = {}
    if debug_taps:
        taps["qt"] = nc.dram_tensor("dbg_qt", [C, LQ], FP8, kind="ExternalOutput").ap()
        taps["kt"] = nc.dram_tensor("dbg_kt", [C, L], FP8, kind="ExternalOutput").ap()
        taps["vt"] = nc.dram_tensor("dbg_vt", [16, 128, 2, 528], FP8, kind="ExternalOutput").ap()
        taps["p80"] = nc.dram_tensor("dbg_p80", [128, 2, 1024], I8, kind="ExternalOutput").ap()
        taps["o00"] = nc.dram_tensor("dbg_o00", [128, 264], F32, kind="ExternalOutput").ap()
        taps["xtT"] = nc.dram_tensor("dbg_xtT", [2, 128, 4, 512], BF16, kind="ExternalOutput").ap()
        taps["xt16"] = nc.dram_tensor("dbg_xt16", [4, 128, LQ], BF16, kind="ExternalOutput").ap()
        taps["xres"] = nc.dram_tensor("dbg_xres", [C, LQ], F32, kind="ExternalOutput").ap()

    with tile.TileContext(nc) as tc, ExitStack() as top:
        persist = top.enter_context(tc.tile_pool(name="persist", bufs=1))
        # PSUM: 4 banks scores/projections + 2 banks O^T + 2 banks out-proj/LN
        ps_s = top.enter_context(tc.tile_pool(name="ps_s", bufs=2, space="PSUM"))
        ps_o = top.enter_context(tc.tile_pool(name="ps_o", bufs=1, space="PSUM"))
        pp = top.enter_context(tc.tile_pool(name="pp", bufs=2, space="PSUM"))
        ptp = top.enter_context(tc.tile_pool(name="ptp", bufs=1))
        nrm = top.enter_context(tc.tile_pool(name="nrm", bufs=2))
        tmp = top.enter_context(tc.tile_pool(name="tmp", bufs=2))
        yout = top.enter_context(tc.tile_pool(name="yout", bufs=2))
        ph1 = top.enter_context(tc.tile_pool(name="ph1", bufs=1))

        def alloc(shape, dt, tag):
            return persist.tile(shape, dt, tag=tag, name=tag)

        # persistent tensors
        pet16 = [alloc([128, LQ], BF16, f"pet16_{i}") for i in range(NC4)]
        wp16 = [alloc([128, C], BF16, f"wp16_{i}") for i in range(NC4)]
        gamma = [alloc([128, 1], F32, f"g_{i}") for i in range(NC4)]
        beta = [alloc([128, 1], F32, f"b_{i}") for i in range(NC4)]
        bp = [alloc([128, 1], F32, f"bp_{i}") for i in range(NC4)]

        qt8 = [alloc([128, LQ], FP8, f"qt8_{i}") for i in range(NH // 2)]
        kt8 = [alloc([128, L], FP8, f"kt8_{i}") for i in range(NH // 2)]
        # 66 cols per head: 64 dims + ones (denominator) + zero pad (dual-fp8
        # ldweights requires an even stationary free dim per k-tile)
        v8 = [alloc([128, 2, 528], FP8, f"v8_{i}") for i in range(NDM)]
        # xtT[lc]: [q 128, qb 4, (h d) 512] bf16 — normalized attention out,
        # query-major.  xt16: [hd 128, q LQ] bf16 — transposed back for the
        # output projection.
        xtT = [alloc([128, NQB, 512], BF16, f"xtT_{lc}") for lc in range(NLQ)]
        xt16 = [alloc([128, LQ], BF16, f"xt16_{i}") for i in range(NC4)]
        xres = [alloc([128, LQ], F32R, f"xr_{i}") for i in range(NC4)]

        # constants
        ones_f = alloc([128, 128], F32, "ones_f")
        onesr = alloc([128, 128], F32R, "onesr")
        nc.vector.memset(ones_f[:], 1.0)
        nc.vector.tensor_copy(onesr[:], ones_f[:])
        bias_t = alloc([128, 1], F32, "bias_t")
        nc.vector.memset(bias_t[:], BIAS_ACT)
        eps_t = alloc([128, 1], F32, "eps_t")
        nc.vector.memset(eps_t[:], LN_EPS)

        # ---- input DMAs (need-order) ---------------------------------------
        def p1load(ap_dram, shape, dt, tag):
            t = ph1.tile(shape, dt, tag=tag, name=tag)
            nc.sync.dma_start(t[:], ap_dram)
            return t
        wq8 = [p1load(wq8_d[t], [128, 2, C], FP8, f"wq8_{t}") for t in range(2)]
        pet8 = [p1load(pet8_d[t], [128, 2, LQ], FP8, f"pet8_{t}") for t in range(2)]
        wk8 = [p1load(wk8_d[t], [128, 2, C], FP8, f"wk8_{t}") for t in range(2)]
        ct8 = [p1load(ct8_d[t], [128, 2, L], FP8, f"ct8_{t}") for t in range(2)]
        wv8 = [p1load(wv8_d[t], [128, 2, C], FP8, f"wv8_{t}") for t in range(2)]
        for i in range(NC4):
            nc.sync.dma_start(wp16[i][:], wp16_d[i])
            nc.sync.dma_start(pet16[i][:], pet16_d[i * 128:(i + 1) * 128, :])
            nc.sync.dma_start(gamma[i][:], gamma_d[i * 128:(i + 1) * 128, :])
            nc.sync.dma_start(beta[i][:], beta_d[i * 128:(i + 1) * 128, :])
            nc.sync.dma_start(bp[i][:], bp_d[i * 128:(i + 1) * 128, :])

        # ---- projections (phase 1, mostly deferred into tasks) -------------
        def sps():
            return ps_s.tile([128, 1024], F32, tag="sAB", name="sp")

        def qproj(pair):
            ps = sps()
            for h in range(2):
                for t in range(2):
                    nc.tensor.matmul(
                        ps[:, h * 512:(h + 1) * 512],
                        wq8[t][:, :, pair * 128:(pair + 1) * 128],
                        pet8[t][:, :, h * 512:(h + 1) * 512],
                        start=(t == 0), stop=(t == 1), perf_mode=DR)
            e_copy(qt8[pair][:], ps[:], 1024)

        def kproj(pair, kcw):
            ps = sps()
            for h in range(2):
                sl = slice(kcw * 1024 + h * 512, kcw * 1024 + (h + 1) * 512)
                for t in range(2):
                    nc.tensor.matmul(
                        ps[:, h * 512:(h + 1) * 512],
                        wk8[t][:, :, pair * 128:(pair + 1) * 128],
                        ct8[t][:, :, sl],
                        start=(t == 0), stop=(t == 1), perf_mode=DR)
            e_copy(kt8[pair][:, kcw * 1024:(kcw + 1) * 1024], ps[:], 1024)

        def vproj(dm):
            ps = sps()
            for j in range(2):
                m = 2 * dm + j
                for t in range(2):
                    nc.tensor.matmul(
                        ps[:, j * 512:(j + 1) * 512],
                        ct8[t][:, :, m * 128:(m + 1) * 128], wv8[t][:],
                        start=(t == 0), stop=(t == 1), perf_mode=DR)
            dst = v8[dm][:].rearrange("p two (h d) -> p two h d", h=NH)[:, :, :, 0:HD]
            src = ps[:].rearrange("p (two h d) -> p two h d", two=2, h=NH)
            e_copy(dst, src, 1024)
            vv = v8[dm][:].rearrange("p two (h d) -> p two h d", h=NH)
            nc.gpsimd.tensor_copy(vv[:, :, :, HD:HD + 1], ones_f[:, 0:1]
                                  .unsqueeze(1).unsqueeze(1)
                                  .broadcast_to((128, 2, NH, 1)))
            bal.busy["gps"] += 150.0
            nc.gpsimd.memset(vv[:, :, :, HD + 1:HD + 2], 0.0)
            bal.busy["gps"] += 130.0

        # ---- task queue ----------------------------------------------------
        tasks = []

        def pop_task():
            if tasks:
                tasks.pop(0)()

        # eager work: everything attention(pair0, lc0) needs up front
        qproj(0)
        for kcw in range(L // 1024):
            kproj(0, kcw)
        for dm in range(4):
            vproj(dm)

        # deferred phase-1: keep V ahead of PV consumption, then later pairs
        for dm in range(4, 8):
            tasks.append(lambda dm=dm: vproj(dm))
        for kcw in range(L // 1024):
            tasks.append(lambda kcw=kcw: kproj(1, kcw))
        tasks.append(lambda: qproj(1))
        for dm in range(8, 12):
            tasks.append(lambda dm=dm: vproj(dm))
        for kcw in range(L // 1024):
            tasks.append(lambda kcw=kcw: kproj(2, kcw))
        tasks.append(lambda: qproj(2))
        for dm in range(12, 16):
            tasks.append(lambda dm=dm: vproj(dm))
        for kcw in range(L // 1024):
            tasks.append(lambda kcw=kcw: kproj(3, kcw))
        tasks.append(lambda: qproj(3))

        # ---- attention -----------------------------------------------------
        def attention(pair, lc):
            hA, hB = 2 * pair, 2 * pair + 1
            oA = ps_o.tile([128, NQB * 66], F32, tag="oA", name="oA")
            oB = ps_o.tile([128, NQB * 66], F32, tag="oB", name="oB")

            def emit_pv(dm, p8t):
                # PSUM accumulation groups are tracked per 2KB bank (the
                # "zero region"): only the first matmul into each O^T tile
                # starts the group (lazily zeroing the bank) and only the
                # last one stops it; the qb=1..3 dm=0 matmuls overwrite
                # their still-pending-zero byte ranges.
                for h, o in ((hA, oA), (hB, oB)):
                    hl = (h % 2) * 512
                    for qb in range(NQB):
                        nc.tensor.matmul(
                            o[:, qb * 66:qb * 66 + 66],
                            p8t[:, :, hl + qb * 128:hl + (qb + 1) * 128],
                            v8[dm][:, :, h * 66:h * 66 + 66],
                            start=(dm == 0 and qb == 0),
                            stop=(dm == NDM - 1 and qb == NQB - 1),
                            perf_mode=DR)

            pend = None   # (dm, p8t): PV delayed one dm so PE never waits exp
            for dm in range(NDM):
                p8t = ptp.tile([128, 2, 1024], FP8, tag="p8", bufs=4, name="p8")
                for j in range(2):
                    m = 2 * dm + j
                    sAB = sps()
                    for h, base in ((0, 0), (1, 64)):
                        nc.tensor.matmul(
                            sAB[:, h * 512:(h + 1) * 512],
                            dr0(kt8[pair][base:base + 64, m * 128:(m + 1) * 128]),
                            dr0(qt8[pair][base:base + 64, lc * 512:(lc + 1) * 512]),
                            perf_mode=DR)
                    which = bal.pick(1024)
                    dst = p8t[:, j, :]
                    if which == "act":
                        nc.scalar.activation(
                            dst, sAB[:], mybir.ActivationFunctionType.Exp,
                            scale=SC_ACT, bias=bias_t[:])
                    else:
                        eng[which].tensor_scalar(
                            dst.bitcast(I8), sAB[:], B8EFF, 0.0,
                            AluOpType.add, AluOpType.max)
                    if pend is not None and j == 1:
                        emit_pv(*pend)
                        pend = None
                    pop_task()
                if debug_taps and (pair, lc, dm) == (0, 0, 0):
                    nc.sync.dma_start(taps["p80"], p8t[:].bitcast(I8))
                pend = (dm, p8t)
            emit_pv(*pend)
            # normalize: xtT[q, (h d)] = O^T[q, d] / den[q]  (den = col 64).
            # The DVE TT ALU has no divide and hw allows only one non-scalar
            # PSUM input, so: reciprocal PSUM->SBUF (tiny), then TT mult with
            # a stride-0 broadcast of the reciprocal row.
            for h, o in ((hA, oA), (hB, oB)):
                ov = o[:].rearrange("p (qb c) -> p qb c", qb=NQB)
                den = nrm.tile([128, NQB], F32, tag="den", bufs=4, name="den")
                bal.pick(NQB, force="dve")
                nc.vector.reciprocal(den[:], ov[:, :, HD:HD + 1])
                bal.pick(NQB * HD, force="dve")
                nc.vector.tensor_tensor(
                    xtT[lc][:, :, h * HD:(h + 1) * HD], ov[:, :, 0:HD],
                    den[:].unsqueeze(2).broadcast_to((128, NQB, HD)),
                    AluOpType.mult)
            if debug_taps and (pair, lc) == (0, 0):
                nc.sync.dma_start(taps["o00"], oA[:])

        # ---- post-attention per-lc work ------------------------------------
        def transpose_block(lc, qb, kc):
            nc.sync.dma_start(
                xt16[kc][:, lc * 512 + qb * 128:lc * 512 + (qb + 1) * 128],
                xtT[lc][:, qb, kc * 128:(kc + 1) * 128],
                transpose=True)

        def proj_chunk(lc, it):
            sl = slice(lc * 512, (lc + 1) * 512)
            ps = pp.tile([128, 512], F32, tag="pp", name="psy")
            for kc in range(NC4):
                nc.tensor.matmul(ps[:], wp16[kc][:, it * 128:(it + 1) * 128],
                                 xt16[kc][:, sl],
                                 start=(kc == 0), stop=(kc == NC4 - 1))
            # xres = (y + bp) + petT (reads PSUM -> DVE)
            bal.pick(512, force="dve")
            nc.vector.scalar_tensor_tensor(
                xres[it][:, sl], ps[:], bp[it][:], pet16[it][:, sl],
                AluOpType.add, AluOpType.add)

        stats = {}

        def ln_sum_chunk(lc):
            sl = slice(lc * 512, (lc + 1) * 512)
            psum = pp.tile([1, 512], F32, tag="pp", name="psum_sum")
            for c in range(NC4):
                nc.tensor.matmul(psum[:], onesr[:, 0:1], xres[c][:, sl],
                                 start=(c == 0), stop=(c == NC4 - 1))
            stats[("sum", lc)] = psum

        def ln_sq_chunk(lc):
            sl = slice(lc * 512, (lc + 1) * 512)
            psq = pp.tile([1, 512], F32, tag="pp", name="psum_sq")
            for c in range(NC4):
                xsq = tmp.tile([128, 512], F32R, tag="xsq", name="xsq")
                bal.pick(512, force="gps", tt=True)
                nc.gpsimd.tensor_tensor(xsq[:], xres[c][:, sl],
                                        xres[c][:, sl], AluOpType.mult)
                nc.tensor.matmul(psq[:], onesr[:, 0:1], xsq[:],
                                 start=(c == 0), stop=(c == NC4 - 1))
            stats[("sq", lc)] = psq

        def ln_rows_chunk(lc):
            psum = stats.pop(("sum", lc))
            psq = stats.pop(("sq", lc))
            mrow = nrm.tile([1, 512], F32R, tag=f"mu{lc}", name=f"mu{lc}")
            m2 = nrm.tile([1, 512], F32, tag=f"m2{lc}", name=f"m2{lc}")
            ve = nrm.tile([1, 512], F32, tag=f"ve{lc}", name=f"ve{lc}")
            stdr = nrm.tile([1, 512], F32R, tag=f"sd{lc}", name=f"sd{lc}")
            bal.pick(512, force="dve")
            nc.vector.tensor_scalar(mrow[:], psum[:], 1.0 / C, None,
                                    AluOpType.mult)
            bal.pick(512, force="gps", tt=True)
            nc.gpsimd.tensor_tensor(m2[:], mrow[:], mrow[:], AluOpType.mult)
            bal.pick(512, force="dve")
            nc.vector.scalar_tensor_tensor(ve[:], psq[:], 1.0 / C, m2[:],
                                           AluOpType.mult, AluOpType.subtract)
            sdf = nrm.tile([1, 512], F32, tag=f"sf{lc}", name=f"sf{lc}")
            bal.pick(512, force="act")
            nc.scalar.activation(sdf[:], ve[:],
                                 mybir.ActivationFunctionType.Sqrt,
                                 bias=eps_t[0:1, :])
            bal.pick(512, force="dve", psum=False)
            with nc.allow_low_precision(reason="f32r view of f32 reciprocal"):
                nc.vector.reciprocal(stdr[:], sdf[:])
            bmu = pp.tile([128, 512], F32, tag="pp", name="bmu")
            bsd = pp.tile([128, 512], F32, tag="pp", name="bsd")
            nc.tensor.matmul(bmu[:], onesr[0:1, :], mrow[:])
            nc.tensor.matmul(bsd[:], onesr[0:1, :], stdr[:])
            # stage broadcasts to SBUF so GPS can run the apply ops
            smu = nrm.tile([128, 512], F32, tag="smu", name="smu")
            ssd = nrm.tile([128, 512], F32, tag="ssd", name="ssd")
            e_copy(smu[:], bmu[:], 512)
            e_copy(ssd[:], bsd[:], 512)
            stats[lc] = (smu, ssd)

        def ln_apply_chunk(lc, c):
            sl = slice(lc * 512, (lc + 1) * 512)
            smu, ssd = stats[lc]
            t = tmp.tile([128, 512], F32, tag="lnt", bufs=2, name="lnt")
            y = yout.tile([128, 512], F32, tag="y", name="yout")
            bal.pick(512, force="gps", tt=True)
            nc.gpsimd.tensor_tensor(t[:], xres[c][:, sl], smu[:],
                                    AluOpType.subtract)
            bal.pick(512, force="gps", tt=True)
            nc.gpsimd.tensor_tensor(t[:], t[:], ssd[:], AluOpType.mult)
            bal.pick(512, force="gps")
            nc.gpsimd.tensor_scalar(y[:], t[:], gamma[c][:], beta[c][:],
                                    AluOpType.mult, AluOpType.add)
            nc.sync.dma_start(out_d[c * 128:(c + 1) * 128, sl], y[:])

        def post_tasks(lc):
            out = []
            for qb in range(NQB):
                for kc in range(NC4):
                    out.append(lambda qb=qb, kc=kc: transpose_block(lc, qb, kc))
            for it in range(NC4):
                out.append(lambda it=it: proj_chunk(lc, it))
            out.append(lambda: ln_sum_chunk(lc))
            out.append(lambda: ln_sq_chunk(lc))
            out.append(lambda: ln_rows_chunk(lc))
            for c in range(NC4):
                out.append(lambda c=c: ln_apply_chunk(lc, c))
            return out

        for lc in range(NLQ):
            for pair in range(NH // 2):
                attention(pair, lc)
            tasks.extend(post_tasks(lc))
        while tasks:
            pop_task()

        if debug_taps:
            for i in range(NH // 2):
                nc.sync.dma_start(taps["qt"][i * 128:(i + 1) * 128, :], qt8[i][:])
                nc.sync.dma_start(taps["kt"][i * 128:(i + 1) * 128, :], kt8[i][:])
            for dm in range(NDM):
                nc.sync.dma_start(taps["vt"][dm], v8[dm][:])
            for lc in range(NLQ):
                nc.sync.dma_start(taps["xtT"][lc], xtT[lc][:])
            for i in range(NC4):
                nc.sync.dma_start(taps["xt16"][i], xt16[i][:])
                nc.sync.dma_start(taps["xres"][i * 128:(i + 1) * 128, :],
                                  xres[i][:].bitcast(F32))

    nc.compile()
    return nc


def prep_core_inputs(inputs):
    """Shard + lay out the full inputs for the 8 cores."""
    pet = np.asarray(inputs["pet_feat"], np.float32).reshape(B, C, L)
    ct = np.asarray(inputs["ct_feat"], np.float32).reshape(B, C, L)
    bf = ml_dtypes.bfloat16

    def wprep(w):
        # [2(t), 128(p), 2(j), 512(out)]: value = W[out, 256t+128j+p]
        wt = np.ascontiguousarray(np.asarray(w, np.float32).T)  # [in, out]
        return np.ascontiguousarray(
            wt.reshape(2, 2, 128, C).transpose(0, 2, 1, 3)).astype(E4)

    wq8 = wprep(inputs["Wq"])
    wk8 = wprep(inputs["Wk"])
    wv8 = wprep(inputs["Wv"])
    # wp16[kc][p][out] = Wp[out, kc*128+p]
    wp16 = np.ascontiguousarray(
        np.asarray(inputs["Wp"], np.float32).T.reshape(NC4, 128, C)).astype(bf)
    gamma = np.asarray(inputs["gamma"], np.float32).reshape(C, 1)
    beta = np.asarray(inputs["beta"], np.float32).reshape(C, 1)
    bp = np.asarray(inputs["bp"], np.float32).reshape(C, 1)

    ct8 = {}
    for b in range(B):
        ct8[b] = np.ascontiguousarray(
            ct[b].reshape(2, 2, 128, L).transpose(0, 2, 1, 3)).astype(E4)

    in_maps = []
    for core in range(NCORES):
        b, jq = divmod(core, 4)
        sl = slice(jq * LQ, (jq + 1) * LQ)
        pet_sl = np.ascontiguousarray(pet[b][:, sl])
        pet8 = np.ascontiguousarray(
            (pet_sl * QPRESCALE).reshape(2, 2, 128, LQ).transpose(0, 2, 1, 3)
        ).astype(E4)
        in_maps.append({
            "pet8": pet8,
            "ct8": ct8[b],
            "pet16": pet_sl.astype(bf),
            "wq8": wq8, "wk8": wk8, "wv8": wv8, "wp16": wp16,
            "gamma": gamma, "beta": beta, "bp": bp,
        })
    return in_maps


def assemble_output(results):
    out = np.empty((B, C, L), np.float32)
    for core in range(NCORES):
        b, jq = divmod(core, 4)
        out[b][:, jq * LQ:(jq + 1) * LQ] = results[core]["out"]
    return out.reshape(B, C, H, W)


_NC_CACHE = {}


def get_nc(debug=False, debug_taps=False):
    key = (debug, debug_taps)
    if key not in _NC_CACHE:
        _NC_CACHE[key] = build_nc(debug=debug, debug_taps=debug_taps)
    return _NC_CACHE[key]


def kernel(**inputs):
    nc = get_nc()
    in_maps = prep_core_inputs(inputs)
    res = bass_utils.run_bass_kernel_spmd(nc, in_maps, list(range(NCORES)))
    return assemble_output(res.results)


# revision 14
# speedup vs baseline: 1.3873x; 1.3873x over previous
# Trainium2 Bass kernel for CrossAttentionFusion — fp8 DoubleRow + transposed
# PV rewrite.
#
# Reference computation (per batch b):
#   pet_seq = pet_feat[b] viewed as (C, L)^T            L = H*W = 4096, C = 512
#   q = pet_seq @ Wq.T ; k = ct_seq @ Wk.T ; v = ct_seq @ Wv.T   (8 heads, hd=64)
#   x = softmax(q k^T / sqrt(hd)) v                      per head
#   y = LN(pet_seq + x @ Wp.T + bp) * gamma + beta       -> (B, C, H, W)
#
# Sharding: 8 cores = 2 batches x 4 query-row chunks (1024 rows each), no
# collectives.
#
# Design notes (v2 — tuned against the TimelineSim cost model):
#   * The kernel is elementwise-bound: the softmax exp must read f32 PSUM and
#     write fp8 SBUF, which only ACT/DVE can do (GPSIMD has no PSUM port, DMA
#     cannot touch PSUM).  All structural choices minimize non-exp ACT/DVE
#     work and keep both engines saturated.
#   * Scores: fp8e4 DoubleRow with a stride-0 broadcast contraction (doubles
#     the product; q side pre-scaled by A8*scale/2 so PSUM = A8*logit).
#   * exp runs split across ACT (table exp) and DVE (Schraudolph int8 bit
#     trick) chosen per-op by a cost-model greedy balancer.
#   * PV runs TRANSPOSED: P8 is the matmul stationary ([128k, 2, 128q]
#     slices), V the moving operand, so O^T lands as [128 q-part, 66] with
#     the softmax denominator (ones column in V) at column 64.  The
#     normalization is then ONE tensor_tensor divide per head with a
#     stride-0 broadcast of the per-partition denominator — no reciprocal
#     chain, no PE broadcasts, no denominator DMAs.
#   * xt is produced in bf16 [q, hd] layout and flipped back to [hd, q] with
#     32 XBAR DMA transposes (14 ns/16x128 tile, runs on the idle DMA path).
#     Out-projection is a plain bf16 matmul (PE has large slack).
#   * LayerNorm: channel sums via ones-column f32r matmuls, row stats on
#     [1,512] strips, apply on GPSIMD (SBUF-only) with gamma/beta as
#     per-partition tensor_scalar.
#   * All projection/copy/post work is queued as micro-tasks popped inside
#     the attention dm-loop so ACT/DVE/PE never drain between phases.

import numpy as np
import ml_dtypes

import concourse.bacc as bacc
import concourse.bass as bass
import concourse.tile as tile
from concourse import mybir
from concourse import bass_utils
from concourse.alu_op_type import AluOpType
from contextlib import ExitStack

F32 = mybir.dt.float32
F32R = mybir.dt.float32r
BF16 = mybir.dt.bfloat16
FP8 = mybir.dt.float8e4
I8 = mybir.dt.int8
E4 = ml_dtypes.float8_e4m3
DR = mybir.MatmulPerfMode.DoubleRow

B, C, H, W = 2, 512, 64, 64
L = H * W                    # 4096
NH, HD = 8, 64
NCORES = 8
LQ = L // 4                  # 1024 query rows per core
ATT_SCALE = HD ** -0.5       # 1/8
LN_EPS = 1e-5

# fp8 Schraudolph constants (e4m3: 4 exp bits bias 7, 3 mantissa bits).
A8 = 8.0 / np.log(2.0)            # octave slope in bit units
B8 = 56.05                        # 7*8 + truncation/Schraudolph tuning
B8EFF = 32.0                      # bias added before int8 trunc
SH = (B8 - B8EFF) / A8            # effective logit shift (~2.084)
QPRESCALE = float(A8 * ATT_SCALE / 2.0)   # pet8 host prescale (DR doubles)
SC_ACT = float(1.0 / A8)
BIAS_ACT = float(-SH)

NC4 = C // 128           # 4 channel chunks of 128
NM = L // 128            # 32 key m-chunks
NDM = NM // 2            # 16 double m-chunks
NLQ = LQ // 512          # 2 query chunks per core
NQB = 4                  # 128-query blocks per lc chunk


def dr0(ap):
    """Stride-0 DoubleRow plane dim: [K, N] -> [K, 2, N] reading data twice."""
    return ap.unsqueeze(1).broadcast_to((ap.shape[0], 2, ap.shape[1]))


class EngineBalancer:
    """Greedy engine assignment using the v2 instruction-cost model.

    engine-busy ns = slope*cols + init/2 where init = 2*access_cycles of the
    slowest operand space.  GPSIMD adds the Q7 launch and runs TT at 0.42
    efficiency.  GPSIMD cannot access PSUM.
    """
    SLOPE = {"act": 0.8333, "dve": 1.0417, "gps": 1.3889}
    INIT_PSUM = {"act": 143.0, "dve": 125.0}
    INIT_SBUF = {"act": 185.0, "dve": 60.0, "gps": 120.0}

    def __init__(self):
        self.busy = {"act": 0.0, "dve": 0.0, "gps": 0.0}

    def cost(self, e, cols, psum=True, tt=False):
        c = self.SLOPE[e] * cols
        if e == "gps":
            if tt:
                c = cols * 0.8333 / 0.42
            return c + self.INIT_SBUF["gps"]
        init = self.INIT_PSUM[e] if psum else self.INIT_SBUF[e]
        return c + init

    def pick(self, cols, force=None, candidates=("act", "dve"), psum=True,
             tt=False):
        if force is None:
            e = min(candidates,
                    key=lambda k: self.busy[k] + self.cost(k, cols, psum, tt))
        else:
            e = force
        self.busy[e] += self.cost(e, cols, psum, tt)
        return e


def build_nc(debug: bool = False, debug_taps: bool = False):
    nc = bacc.Bacc("TRN2", target_bir_lowering=False, debug=debug,
                   num_devices=NCORES)
    eng = {"act": nc.scalar, "dve": nc.vector, "gps": nc.gpsimd}
    bal = EngineBalancer()

    def e_copy(dst, src, cols, force=None, psum=True):
        which = bal.pick(cols, force, psum=psum)
        if which == "act":
            nc.scalar.copy(dst, src)
        else:
            eng[which].tensor_copy(dst, src)

    # ---- DRAM I/O ----------------------------------------------------------
    pet8_d = nc.dram_tensor("pet8", [2, 128, 2, LQ], FP8, kind="ExternalInput").ap()
    ct8_d = nc.dram_tensor("ct8", [2, 128, 2, L], FP8, kind="ExternalInput").ap()
    pet16_d = nc.dram_tensor("pet16", [C, LQ], BF16, kind="ExternalInput").ap()
    wq8_d = nc.dram_tensor("wq8", [2, 128, 2, C], FP8, kind="ExternalInput").ap()
    wk8_d = nc.dram_tensor("wk8", [2, 128, 2, C], FP8, kind="ExternalInput").ap()
    wv8_d = nc.dram_tensor("wv8", [2, 128, 2, C], FP8, kind="ExternalInput").ap()
    wp16_d = nc.dram_tensor("wp16", [NC4, 128, C], BF16, kind="ExternalInput").ap()
    gamma_d = nc.dram_tensor("gamma", [C, 1], F32, kind="ExternalInput").ap()
    beta_d = nc.dram_tensor("beta", [C, 1], F32, kind="ExternalInput").ap()
    bp_d = nc.dram_tensor("bp", [C, 1], F32, kind="ExternalInput").ap()
    out_d = nc.dram_tensor("out", [C, LQ], F32, kind="ExternalOutput").ap()
    taps = {}
    if debug_taps:
        taps["qt"] = nc.dram_tensor("dbg_qt", [C, LQ], FP8, kind="ExternalOutput").ap()
        taps["kt"] = nc.dram_tensor("dbg_kt", [C, L], FP8, kind="ExternalOutput").ap()
        taps["vt"] = nc.dram_tensor("dbg_vt", [16, 128, 2, 528], FP8, kind="ExternalOutput").ap()
        taps["p80"] = nc.dram_tensor("dbg_p80", [128, 2, 1024], I8, kind="ExternalOutput").ap()
        taps["o00"] = nc.dram_tensor("dbg_o00", [128, 264], F32, kind="ExternalOutput").ap()
        taps["xtT"] = nc.dram_tensor("dbg_xtT", [2, 128, 4, 512], BF16, kind="ExternalOutput").ap()
        taps["xt16"] = nc.dram_tensor("dbg_xt16", [4, 128, LQ], BF16, kind="ExternalOutput").ap()
        taps["xres"] = nc.dram_tensor("dbg_xres", [C, LQ], F32, kind="ExternalOutput").ap()

    with tile.TileContext(nc) as tc, ExitStack() as top:
        persist = top.enter_context(tc.tile_pool(name="persist", bufs=1))
        # PSUM during attention: 6 banks scores (3-deep, decouples the
        # exp->scores->exp dependency cycle) + 2 banks O^T.  The out-proj/LN
        # pool opens after these close (post phase).
        att_scope = ExitStack()
        ps_s = att_scope.enter_context(tc.tile_pool(name="ps_s", bufs=3, space="PSUM"))
        ps_o = att_scope.enter_context(tc.tile_pool(name="ps_o", bufs=1, space="PSUM"))
        ptp = top.enter_context(tc.tile_pool(name="ptp", bufs=1))
        nrm = top.enter_context(tc.tile_pool(name="nrm", bufs=2))
        tmp = top.enter_context(tc.tile_pool(name="tmp", bufs=2))
        yout = top.enter_context(tc.tile_pool(name="yout", bufs=2))
        ph1 = top.enter_context(tc.tile_pool(name="ph1", bufs=1))

        def alloc(shape, dt, tag):
            return persist.tile(shape, dt, tag=tag, name=tag)

        # persistent tensors
        pet16 = [alloc([128, LQ], BF16, f"pet16_{i}") for i in range(NC4)]
        wp16 = [alloc([128, C], BF16, f"wp16_{i}") for i in range(NC4)]
        gamma = [alloc([128, 1], F32, f"g_{i}") for i in range(NC4)]
        beta = [alloc([128, 1], F32, f"b_{i}") for i in range(NC4)]
        bp = [alloc([128, 1], F32, f"bp_{i}") for i in range(NC4)]

        qt8 = [alloc([128, LQ], FP8, f"qt8_{i}") for i in range(NH // 2)]
        kt8 = [alloc([128, L], FP8, f"kt8_{i}") for i in range(NH // 2)]
        # 66 cols per head: 64 dims + ones (denominator) + zero pad (dual-fp8
        # ldweights requires an even stationary free dim per k-tile)
        v8 = [alloc([128, 2, 528], FP8, f"v8_{i}") for i in range(NDM)]
        # xtT[lc]: [q 128, qb 4, (h d) 512] bf16 — normalized attention out,
        # query-major.  xt16: [hd 128, q LQ] bf16 — transposed back for the
        # output projection.
        xtT = [alloc([128, NQB, 512], BF16, f"xtT_{lc}") for lc in range(NLQ)]
        xt16 = [alloc([128, LQ], BF16, f"xt16_{i}") for i in range(NC4)]
        xres = [alloc([128, LQ], F32R, f"xr_{i}") for i in range(NC4)]

        # constants
        ones_f = alloc([128, 128], F32, "ones_f")
        onesr = alloc([128, 128], F32R, "onesr")
        nc.vector.memset(ones_f[:], 1.0)
        nc.vector.tensor_copy(onesr[:], ones_f[:])
        bias_t = alloc([128, 1], F32, "bias_t")
        nc.vector.memset(bias_t[:], BIAS_ACT)
        eps_t = alloc([128, 1], F32, "eps_t")
        nc.vector.memset(eps_t[:], LN_EPS)

        # ---- input DMAs (need-order) ---------------------------------------
        def p1load(ap_dram, shape, dt, tag):
            t = ph1.tile(shape, dt, tag=tag, name=tag)
            nc.sync.dma_start(t[:], ap_dram)
            return t
        wq8 = [p1load(wq8_d[t], [128, 2, C], FP8, f"wq8_{t}") for t in range(2)]
        pet8 = [p1load(pet8_d[t], [128, 2, LQ], FP8, f"pet8_{t}") for t in range(2)]
        wk8 = [p1load(wk8_d[t], [128, 2, C], FP8, f"wk8_{t}") for t in range(2)]
        # ct8 loaded in kcw-granular chunks so the first K projection (and
        # thus the first scores matmul) starts ~2us in instead of waiting for
        # the full 2MB
        ct8 = [ph1.tile([128, 2, L], FP8, tag=f"ct8_{t}", name=f"ct8_{t}")
               for t in range(2)]
        for kcw in range(L // 1024):
            for t in range(2):
                nc.sync.dma_start(ct8[t][:, :, kcw * 1024:(kcw + 1) * 1024],
                                  ct8_d[t][:, :, kcw * 1024:(kcw + 1) * 1024])
            if kcw == 0:
                wv8 = [p1load(wv8_d[t], [128, 2, C], FP8, f"wv8_{t}")
                       for t in range(2)]
        for i in range(NC4):
            nc.sync.dma_start(wp16[i][:], wp16_d[i])
            nc.sync.dma_start(pet16[i][:], pet16_d[i * 128:(i + 1) * 128, :])
            nc.sync.dma_start(gamma[i][:], gamma_d[i * 128:(i + 1) * 128, :])
            nc.sync.dma_start(beta[i][:], beta_d[i * 128:(i + 1) * 128, :])
            nc.sync.dma_start(bp[i][:], bp_d[i * 128:(i + 1) * 128, :])

        # ---- projections (phase 1, mostly deferred into tasks) -------------
        def sps():
            return ps_s.tile([128, 1024], F32, tag="sAB", name="sp")

        def qproj(pair):
            ps = sps()
            for h in range(2):
                for t in range(2):
                    nc.tensor.matmul(
                        ps[:, h * 512:(h + 1) * 512],
                        wq8[t][:, :, pair * 128:(pair + 1) * 128],
                        pet8[t][:, :, h * 512:(h + 1) * 512],
                        start=(t == 0), stop=(t == 1), perf_mode=DR)
            e_copy(qt8[pair][:], ps[:], 1024)

        def kproj(pair, kcw):
            ps = sps()
            for h in range(2):
                sl = slice(kcw * 1024 + h * 512, kcw * 1024 + (h + 1) * 512)
                for t in range(2):
                    nc.tensor.matmul(
                        ps[:, h * 512:(h + 1) * 512],
                        wk8[t][:, :, pair * 128:(pair + 1) * 128],
                        ct8[t][:, :, sl],
                        start=(t == 0), stop=(t == 1), perf_mode=DR)
            e_copy(kt8[pair][:, kcw * 1024:(kcw + 1) * 1024], ps[:], 1024)

        def vproj(dm):
            ps = sps()
            for j in range(2):
                m = 2 * dm + j
                for t in range(2):
                    nc.tensor.matmul(
                        ps[:, j * 512:(j + 1) * 512],
                        ct8[t][:, :, m * 128:(m + 1) * 128], wv8[t][:],
                        start=(t == 0), stop=(t == 1), perf_mode=DR)
            dst = v8[dm][:].rearrange("p two (h d) -> p two h d", h=NH)[:, :, :, 0:HD]
            src = ps[:].rearrange("p (two h d) -> p two h d", two=2, h=NH)
            e_copy(dst, src, 1024)
            vv = v8[dm][:].rearrange("p two (h d) -> p two h d", h=NH)
            nc.gpsimd.tensor_copy(vv[:, :, :, HD:HD + 1], ones_f[:, 0:1]
                                  .unsqueeze(1).unsqueeze(1)
                                  .broadcast_to((128, 2, NH, 1)))
            bal.busy["gps"] += 150.0
            nc.gpsimd.memset(vv[:, :, :, HD + 1:HD + 2], 0.0)
            bal.busy["gps"] += 130.0

        # ---- task queues ---------------------------------------------------
        # urgent: must be emitted early next call (divides freeing O^T PSUM,
        #   before the next pair's first PV matmuls are emitted at dm1-j1).
        # tasks: compute-only phase-1 work, popped once per j (no stall risk).
        urgent = []
        tasks = []

        def pop_task(slot):
            # slot: (dm, j)
            dm, j = slot
            if urgent and j == 0 and dm in (0, 1):
                while urgent:
                    urgent.pop(0)()
                return
            if tasks:
                tasks.pop(0)()

        # eager work: everything attention(pair0, lc0) needs up front
        qproj(0)
        for kcw in range(L // 1024):
            kproj(0, kcw)
        for dm in range(4):
            vproj(dm)

        # deferred phase-1: keep V ahead of PV consumption, then later pairs
        for dm in range(4, 8):
            tasks.append(lambda dm=dm: vproj(dm))
        for kcw in range(L // 1024):
            tasks.append(lambda kcw=kcw: kproj(1, kcw))
        tasks.append(lambda: qproj(1))
        for dm in range(8, 12):
            tasks.append(lambda dm=dm: vproj(dm))
        for kcw in range(L // 1024):
            tasks.append(lambda kcw=kcw: kproj(2, kcw))
        tasks.append(lambda: qproj(2))
        for dm in range(12, 16):
            tasks.append(lambda dm=dm: vproj(dm))
        for kcw in range(L // 1024):
            tasks.append(lambda kcw=kcw: kproj(3, kcw))
        tasks.append(lambda: qproj(3))

        # ---- attention -----------------------------------------------------
        def attention(pair, lc):
            hA, hB = 2 * pair, 2 * pair + 1
            oA = ps_o.tile([128, NQB * 66], F32, tag="oA", name="oA")
            oB = ps_o.tile([128, NQB * 66], F32, tag="oB", name="oB")

            def emit_pv(dm, p8t):
                # PSUM accumulation groups are tracked per 2KB bank (the
                # "zero region"): only the first matmul into each O^T tile
                # starts the group (lazily zeroing the bank) and only the
                # last one stops it; the qb=1..3 dm=0 matmuls overwrite
                # their still-pending-zero byte ranges.
                for h, o in ((hA, oA), (hB, oB)):
                    hl = (h % 2) * 512
                    for qb in range(NQB):
                        nc.tensor.matmul(
                            o[:, qb * 66:qb * 66 + 66],
                            p8t[:, :, hl + qb * 128:hl + (qb + 1) * 128],
                            v8[dm][:, :, h * 66:h * 66 + 66],
                            start=(dm == 0 and qb == 0),
                            stop=(dm == NDM - 1 and qb == NQB - 1),
                            perf_mode=DR)

            pend = None   # (dm, p8t): PV delayed one dm so PE never waits exp
            for dm in range(NDM):
                p8t = ptp.tile([128, 2, 1024], FP8, tag="p8", bufs=4, name="p8")
                for j in range(2):
                    m = 2 * dm + j
                    sAB = sps()
                    for h, base in ((0, 0), (1, 64)):
                        nc.tensor.matmul(
                            sAB[:, h * 512:(h + 1) * 512],
                            dr0(kt8[pair][base:base + 64, m * 128:(m + 1) * 128]),
                            dr0(qt8[pair][base:base + 64, lc * 512:(lc + 1) * 512]),
                            perf_mode=DR)
                    which = bal.pick(1024)
                    dst = p8t[:, j, :]
                    if which == "act":
                        nc.scalar.activation(
                            dst, sAB[:], mybir.ActivationFunctionType.Exp,
                            scale=SC_ACT, bias=bias_t[:])
                    else:
                        eng[which].tensor_scalar(
                            dst.bitcast(I8), sAB[:], B8EFF, 0.0,
                            AluOpType.add, AluOpType.max)
                    if pend is not None and j == 1:
                        emit_pv(*pend)
                        pend = None
                    pop_task((dm, j))
                if debug_taps and (pair, lc, dm) == (0, 0, 0):
                    nc.sync.dma_start(taps["p80"], p8t[:].bitcast(I8))
                pend = (dm, p8t)
            emit_pv(*pend)

            # normalize: xtT[q, (h d)] = O^T[q, d] / den[q]  (den = col 64).
            # The DVE TT ALU has no divide and hw allows only one non-scalar
            # PSUM input, so: reciprocal PSUM->SBUF (tiny), then TT mult with
            # a stride-0 broadcast of the reciprocal row.  Deferred (urgent
            # queue) so the next call's exps cover the last PV's latency.
            def divide(h, o):
                ov = o[:].rearrange("p (qb c) -> p qb c", qb=NQB)
                den = nrm.tile([128, NQB], F32, tag="den", bufs=4, name="den")
                bal.pick(NQB, force="dve")
                nc.vector.reciprocal(den[:], ov[:, :, HD:HD + 1])
                bal.pick(NQB * HD, force="dve")
                nc.vector.tensor_tensor(
                    xtT[lc][:, :, h * HD:(h + 1) * HD], ov[:, :, 0:HD],
                    den[:].unsqueeze(2).broadcast_to((128, NQB, HD)),
                    AluOpType.mult)
                if debug_taps and (pair, lc) == (0, 0) and h == hA:
                    nc.sync.dma_start(taps["o00"], o[:])

            def div_and_transpose():
                divide(hA, oA)
                divide(hB, oB)
                # the 4 transpose blocks of column-pair `pair` only need
                # this pair's divides (pure DMA, scheduler places them)
                for qb in range(NQB):
                    transpose_block(lc, qb, pair)

            urgent.append(div_and_transpose)

        # ---- post-attention per-lc work ------------------------------------
        def transpose_block(lc, qb, kc):
            nc.sync.dma_start(
                xt16[kc][:, lc * 512 + qb * 128:lc * 512 + (qb + 1) * 128],
                xtT[lc][:, qb, kc * 128:(kc + 1) * 128],
                transpose=True)

        def proj_chunk(lc, it):
            sl = slice(lc * 512, (lc + 1) * 512)
            ps = pp.tile([128, 512], F32, tag="pp", name="psy")
            for kc in range(NC4):
                nc.tensor.matmul(ps[:], wp16[kc][:, it * 128:(it + 1) * 128],
                                 xt16[kc][:, sl],
                                 start=(kc == 0), stop=(kc == NC4 - 1))
            # xres = (y + bp) + petT (reads PSUM -> DVE)
            bal.pick(512, force="dve")
            nc.vector.scalar_tensor_tensor(
                xres[it][:, sl], ps[:], bp[it][:], pet16[it][:, sl],
                AluOpType.add, AluOpType.add)

        stats = {}

        def ln_sum_chunk(lc):
            sl = slice(lc * 512, (lc + 1) * 512)
            psum = pp.tile([1, 512], F32, tag="pp", name="psum_sum")
            for c in range(NC4):
                nc.tensor.matmul(psum[:], onesr[:, 0:1], xres[c][:, sl],
                                 start=(c == 0), stop=(c == NC4 - 1))
            stats[("sum", lc)] = psum

        def ln_sq_chunk(lc):
            sl = slice(lc * 512, (lc + 1) * 512)
            psq = pp.tile([1, 512], F32, tag="pp", name="psum_sq")
            for c in range(NC4):
                xsq = tmp.tile([128, 512], F32R, tag="xsq", name="xsq")
                e = bal.pick(512, candidates=("dve", "gps"), psum=False,
                             tt=True)
                eng[e].tensor_tensor(xsq[:], xres[c][:, sl],
                                     xres[c][:, sl], AluOpType.mult)
                nc.tensor.matmul(psq[:], onesr[:, 0:1], xsq[:],
                                 start=(c == 0), stop=(c == NC4 - 1))
            stats[("sq", lc)] = psq

        def ln_rows_chunk(lc):
            psum = stats.pop(("sum", lc))
            psq = stats.pop(("sq", lc))
            mrow = nrm.tile([1, 512], F32R, tag=f"mu{lc}", name=f"mu{lc}")
            m2 = nrm.tile([1, 512], F32, tag=f"m2{lc}", name=f"m2{lc}")
            ve = nrm.tile([1, 512], F32, tag=f"ve{lc}", name=f"ve{lc}")
            stdr = nrm.tile([1, 512], F32R, tag=f"sd{lc}", name=f"sd{lc}")
            bal.pick(512, force="dve")
            nc.vector.tensor_scalar(mrow[:], psum[:], 1.0 / C, None,
                                    AluOpType.mult)
            bal.pick(512, force="gps", tt=True)
            nc.gpsimd.tensor_tensor(m2[:], mrow[:], mrow[:], AluOpType.mult)
            bal.pick(512, force="dve")
            nc.vector.scalar_tensor_tensor(ve[:], psq[:], 1.0 / C, m2[:],
                                           AluOpType.mult, AluOpType.subtract)
            sdf = nrm.tile([1, 512], F32, tag=f"sf{lc}", name=f"sf{lc}")
            bal.pick(512, force="act")
            nc.scalar.activation(sdf[:], ve[:],
                                 mybir.ActivationFunctionType.Sqrt,
                                 bias=eps_t[0:1, :])
            bal.pick(512, force="dve", psum=False)
            with nc.allow_low_precision(reason="f32r view of f32 reciprocal"):
                nc.vector.reciprocal(stdr[:], sdf[:])
            bmu = pp.tile([128, 512], F32, tag="pp", name="bmu")
            bsd = pp.tile([128, 512], F32, tag="pp", name="bsd")
            nc.tensor.matmul(bmu[:], onesr[0:1, :], mrow[:])
            nc.tensor.matmul(bsd[:], onesr[0:1, :], stdr[:])
            # stage broadcasts to SBUF so GPS can run the apply ops
            smu = nrm.tile([128, 512], F32, tag="smu", name="smu")
            ssd = nrm.tile([128, 512], F32, tag="ssd", name="ssd")
            e_copy(smu[:], bmu[:], 512)
            e_copy(ssd[:], bsd[:], 512)
            stats[lc] = (smu, ssd)

        def ln_apply_chunk(lc, c):
            sl = slice(lc * 512, (lc + 1) * 512)
            smu, ssd = stats[lc]
            t = tmp.tile([128, 512], F32, tag="lnt", bufs=2, name="lnt")
            y = yout.tile([128, 512], F32, tag="y", name="yout")
            # whole chunk on one engine (DVE or GPSIMD), balancer's choice
            cost = {"dve": 3 * (512 * 1.0417 + 60.0),
                    "gps": 2 * (512 * 0.8333 / 0.42 + 120.0)
                    + 512 * 1.3889 + 120.0}
            e = min(("dve", "gps"), key=lambda k: bal.busy[k] + cost[k])
            bal.busy[e] += cost[e]
            eng[e].tensor_tensor(t[:], xres[c][:, sl], smu[:],
                                 AluOpType.subtract)
            eng[e].tensor_tensor(t[:], t[:], ssd[:], AluOpType.mult)
            eng[e].tensor_scalar(y[:], t[:], gamma[c][:], beta[c][:],
                                 AluOpType.mult, AluOpType.add)
            nc.sync.dma_start(out_d[c * 128:(c + 1) * 128, sl], y[:])

        for lc in range(NLQ):
            for pair in range(NH // 2):
                attention(pair, lc)
        while urgent:
            urgent.pop(0)()
        while tasks:
            tasks.pop(0)()

        # ---- post phase: out-projection + LayerNorm ------------------------
        # The attention PSUM pools close here; the post pool takes over their
        # banks.  The tile scheduler orders by readiness, so these chunks
        # pipeline across engines (PE proj -> DVE residual -> stats -> apply).
        att_scope.close()
        pp = top.enter_context(tc.tile_pool(name="pp", bufs=4, space="PSUM"))
        bal.busy = {"act": 0.0, "dve": 0.0, "gps": 0.0}
        for lc in range(NLQ):
            for it in range(NC4):
                proj_chunk(lc, it)
        for lc in range(NLQ):
            ln_sum_chunk(lc)
            ln_sq_chunk(lc)
            ln_rows_chunk(lc)
        for lc in range(NLQ):
            for c in range(NC4):
                ln_apply_chunk(lc, c)

        if debug_taps:
            for i in range(NH // 2):
                nc.sync.dma_start(taps["qt"][i * 128:(i + 1) * 128, :], qt8[i][:])
                nc.sync.dma_start(taps["kt"][i * 128:(i + 1) * 128, :], kt8[i][:])
            for dm in range(NDM):
                nc.sync.dma_start(taps["vt"][dm], v8[dm][:])
            for lc in range(NLQ):
                nc.sync.dma_start(taps["xtT"][lc], xtT[lc][:])
            for i in range(NC4):
                nc.sync.dma_start(taps["xt16"][i], xt16[i][:])
                nc.sync.dma_start(taps["xres"][i * 128:(i + 1) * 128, :],
                                  xres[i][:].bitcast(F32))

    nc.compile()
    return nc


def prep_core_inputs(inputs):
    """Shard + lay out the full inputs for the 8 cores."""
    pet = np.asarray(inputs["pet_feat"], np.float32).reshape(B, C, L)
    ct = np.asarray(inputs["ct_feat"], np.float32).reshape(B, C, L)
    bf = ml_dtypes.bfloat16

    def wprep(w):
        # [2(t), 128(p), 2(j), 512(out)]: value = W[out, 256t+128j+p]
        wt = np.ascontiguousarray(np.asarray(w, np.float32).T)  # [in, out]
        return np.ascontiguousarray(
            wt.reshape(2, 2, 128, C).transpose(0, 2, 1, 3)).astype(E4)

    wq8 = wprep(inputs["Wq"])
    wk8 = wprep(inputs["Wk"])
    wv8 = wprep(inputs["Wv"])
    # wp16[kc][p][out] = Wp[out, kc*128+p]
    wp16 = np.ascontiguousarray(
        np.asarray(inputs["Wp"], np.float32).T.reshape(NC4, 128, C)).astype(bf)
    gamma = np.asarray(inputs["gamma"], np.float32).reshape(C, 1)
    beta = np.asarray(inputs["beta"], np.float32).reshape(C, 1)
    bp = np.asarray(inputs["bp"], np.float32).reshape(C, 1)

    ct8 = {}
    for b in range(B):
        ct8[b] = np.ascontiguousarray(
            ct[b].reshape(2, 2, 128, L).transpose(0, 2, 1, 3)).astype(E4)

    in_maps = []
    for core in range(NCORES):
        b, jq = divmod(core, 4)
        sl = slice(jq * LQ, (jq + 1) * LQ)
        pet_sl = np.ascontiguousarray(pet[b][:, sl])
        pet8 = np.ascontiguousarray(
            (pet_sl * QPRESCALE).reshape(2, 2, 128, LQ).transpose(0, 2, 1, 3)
        ).astype(E4)
        in_maps.append({
            "pet8": pet8,
            "ct8": ct8[b],
            "pet16": pet_sl.astype(bf),
            "wq8": wq8, "wk8": wk8, "wv8": wv8, "wp16": wp16,
            "gamma": gamma, "beta": beta, "bp": bp,
        })
    return in_maps


def assemble_output(results):
    out = np.empty((B, C, L), np.float32)
    for core in range(NCORES):
        b, jq = divmod(core, 4)
        out[b][:, jq * LQ:(jq + 1) * LQ] = results[core]["out"]
    return out.reshape(B, C, H, W)


_NC_CACHE = {}


def get_nc(debug=False, debug_taps=False):
    key = (debug, debug_taps)
    if key not in _NC_CACHE:
        _NC_CACHE[key] = build_nc(debug=debug, debug_taps=debug_taps)
    return _NC_CACHE[key]


def kernel(**inputs):
    nc = get_nc()
    in_maps = prep_core_inputs(inputs)
    res = bass_utils.run_bass_kernel_spmd(nc, in_maps, list(range(NCORES)))
    return assemble_output(res.results)


# revision 49
# speedup vs baseline: 1.4373x; 1.0360x over previous
# Trainium2 Bass kernel for CrossAttentionFusion — fp8 DoubleRow + transposed
# PV rewrite.
#
# Reference computation (per batch b):
#   pet_seq = pet_feat[b] viewed as (C, L)^T            L = H*W = 4096, C = 512
#   q = pet_seq @ Wq.T ; k = ct_seq @ Wk.T ; v = ct_seq @ Wv.T   (8 heads, hd=64)
#   x = softmax(q k^T / sqrt(hd)) v                      per head
#   y = LN(pet_seq + x @ Wp.T + bp) * gamma + beta       -> (B, C, H, W)
#
# Sharding: 8 cores = 2 batches x 4 query-row chunks (1024 rows each), no
# collectives.
#
# Design notes (v2 — tuned against the TimelineSim cost model; 310us -> 226.7us):
#   * The kernel is elementwise-bound: the softmax exp must read f32 PSUM and
#     write fp8 SBUF, which only ACT/DVE can do (GPSIMD has no PSUM port and
#     DMA cannot touch PSUM — both verified against the walrus verifier).
#     All structural choices minimize non-exp ACT/DVE work and keep both
#     engines saturated (~90% busy through the attention phase).
#   * Scores: fp8e4 DoubleRow with a stride-0 broadcast contraction (doubles
#     the product; q side pre-scaled by A8*scale/2 so PSUM = A8*logit).
#   * exp runs split across ACT (table exp) and DVE (Schraudolph int8 bit
#     trick) chosen per-op by a cost-model greedy balancer.  The scores PSUM
#     pool is 3 tiles deep: with only 2, the exp->scores->exp dependency
#     cycle (~1.7us) caps exp issue at ~62% of engine capacity.
#   * PV runs TRANSPOSED: P8 is the matmul stationary ([128k, 2, 128q]
#     slices), V the moving operand, so O^T lands as [128 q-part, 4qb x 66]
#     with the softmax denominator (ones column in V) at column 64.  PSUM
#     accumulation groups are bank-granular: only the first matmul into each
#     O^T tile sets start (lazy-zeroing the bank), only the last sets stop.
#     Normalization is then per-PARTITION: one tiny reciprocal plus one
#     broadcast tensor_tensor mult per head (the DVE TT ALU has no divide,
#     and only one non-scalar PSUM input is allowed per instruction) — no
#     denominator DMAs or PE broadcasts.  Divides are deferred to the next
#     call's dm1 slot (urgent queue) so the busier exp stream covers the
#     last PV matmul's latency; they must be emitted before the next pair's
#     first PV (PV trails the exp stream by two dm) because the O^T pool is
#     1-deep.
#   * xt is produced in bf16 [q, hd] layout and flipped back to [hd, q] with
#     32 XBAR DMA transposes (14 ns/16x128 tile, on the otherwise idle DMA
#     path), emitted right after each pair's divides.  Out-projection is a
#     plain bf16 matmul (PE has large slack).
#   * Phase-1 (Q/K/V projections + fp8 copies) shares the scores PSUM pool
#     and is drip-fed into the attention stream as micro-tasks (one per j)
#     so ACT/DVE never drain; weight/pet loads are single DMAs spanning both
#     contraction halves (HWDGE charges ~625ns per DMA instruction) and ct8
#     is kcw-chunked, so the first scores start ~2us in.
#   * lc0's out-proj/LN pipeline runs inside lc1's attention calls,
#     borrowing one scores-pool PSUM slot at a time; lc1's runs as a post
#     phase whose PSUM pool reuses the attention banks (pool aliasing blocks
#     any earlier start).  LN: channel sums via ones-column f32r matmuls, row stats
#     on [1,512] strips (Sqrt on ACT — Exp and Sqrt live in different ACT
#     table sets, so the one unavoidable 1.3us table swap happens here),
#     mean/istd broadcast via K=1 matmuls, apply split [sub+mult on DVE or
#     GPSIMD, gamma/beta as per-partition scale+bias on whichever of
#     ACT/DVE/GPSIMD is ahead].

import numpy as np
import ml_dtypes

import concourse.bacc as bacc
import concourse.bass as bass
import concourse.tile as tile
from concourse import mybir
from concourse import bass_utils
from concourse.alu_op_type import AluOpType
from contextlib import ExitStack

F32 = mybir.dt.float32
F32R = mybir.dt.float32r
BF16 = mybir.dt.bfloat16
FP8 = mybir.dt.float8e4
I8 = mybir.dt.int8
E4 = ml_dtypes.float8_e4m3
DR = mybir.MatmulPerfMode.DoubleRow

B, C, H, W = 2, 512, 64, 64
L = H * W                    # 4096
NH, HD = 8, 64
NCORES = 8
LQ = L // 4                  # 1024 query rows per core
ATT_SCALE = HD ** -0.5       # 1/8
LN_EPS = 1e-5

# fp8 Schraudolph constants (e4m3: 4 exp bits bias 7, 3 mantissa bits).
A8 = 8.0 / np.log(2.0)            # octave slope in bit units
B8 = 56.05                        # 7*8 + truncation/Schraudolph tuning
B8EFF = 32.0                      # bias added before int8 trunc
SH = (B8 - B8EFF) / A8            # effective logit shift (~2.084)
QPRESCALE = float(A8 * ATT_SCALE / 2.0)   # pet8 host prescale (DR doubles)
SC_ACT = float(1.0 / A8)
BIAS_ACT = float(-SH)

NC4 = C // 128           # 4 channel chunks of 128
NM = L // 128            # 32 key m-chunks
NDM = NM // 2            # 16 double m-chunks
NLQ = LQ // 512          # 2 query chunks per core
NQB = 4                  # 128-query blocks per lc chunk


def dr0(ap):
    """Stride-0 DoubleRow plane dim: [K, N] -> [K, 2, N] reading data twice."""
    return ap.unsqueeze(1).broadcast_to((ap.shape[0], 2, ap.shape[1]))


class EngineBalancer:
    """Greedy engine assignment using the v2 instruction-cost model.

    engine-busy ns = slope*cols + init/2 where init = 2*access_cycles of the
    slowest operand space.  GPSIMD adds the Q7 launch and runs TT at 0.42
    efficiency.  GPSIMD cannot access PSUM.
    """
    SLOPE = {"act": 0.8333, "dve": 1.0417, "gps": 1.3889}
    INIT_PSUM = {"act": 143.0, "dve": 125.0}
    INIT_SBUF = {"act": 185.0, "dve": 60.0, "gps": 120.0}

    def __init__(self):
        self.busy = {"act": 0.0, "dve": 0.0, "gps": 0.0}

    def cost(self, e, cols, psum=True, tt=False):
        c = self.SLOPE[e] * cols
        if e == "gps":
            if tt:
                c = cols * 0.8333 / 0.42
            return c + self.INIT_SBUF["gps"]
        init = self.INIT_PSUM[e] if psum else self.INIT_SBUF[e]
        return c + init

    def pick(self, cols, force=None, candidates=("act", "dve"), psum=True,
             tt=False):
        if force is None:
            e = min(candidates,
                    key=lambda k: self.busy[k] + self.cost(k, cols, psum, tt))
        else:
            e = force
        self.busy[e] += self.cost(e, cols, psum, tt)
        return e


def build_nc(debug: bool = False, debug_taps: bool = False):
    nc = bacc.Bacc("TRN2", target_bir_lowering=False, debug=debug,
                   num_devices=NCORES)
    eng = {"act": nc.scalar, "dve": nc.vector, "gps": nc.gpsimd}
    bal = EngineBalancer()

    def e_copy(dst, src, cols, force=None, psum=True):
        which = bal.pick(cols, force, psum=psum)
        if which == "act":
            nc.scalar.copy(dst, src)
        else:
            eng[which].tensor_copy(dst, src)

    # ---- DRAM I/O ----------------------------------------------------------
    pet8_d = nc.dram_tensor("pet8", [2, 128, 2, LQ], FP8, kind="ExternalInput").ap()
    ct8_d = nc.dram_tensor("ct8", [2, 128, 2, L], FP8, kind="ExternalInput").ap()
    pet16_d = nc.dram_tensor("pet16", [C, LQ], BF16, kind="ExternalInput").ap()
    wq8_d = nc.dram_tensor("wq8", [2, 128, 2, C], FP8, kind="ExternalInput").ap()
    wk8_d = nc.dram_tensor("wk8", [2, 128, 2, C], FP8, kind="ExternalInput").ap()
    wv8_d = nc.dram_tensor("wv8", [2, 128, 2, C], FP8, kind="ExternalInput").ap()
    wp16_d = nc.dram_tensor("wp16", [NC4, 128, C], BF16, kind="ExternalInput").ap()
    gamma_d = nc.dram_tensor("gamma", [C, 1], F32, kind="ExternalInput").ap()
    beta_d = nc.dram_tensor("beta", [C, 1], F32, kind="ExternalInput").ap()
    bp_d = nc.dram_tensor("bp", [C, 1], F32, kind="ExternalInput").ap()
    out_d = nc.dram_tensor("out", [C, LQ], F32, kind="ExternalOutput").ap()
    taps = {}
    if debug_taps:
        taps["qt"] = nc.dram_tensor("dbg_qt", [C, LQ], FP8, kind="ExternalOutput").ap()
        taps["kt"] = nc.dram_tensor("dbg_kt", [C, L], FP8, kind="ExternalOutput").ap()
        taps["vt"] = nc.dram_tensor("dbg_vt", [16, 128, 2, 528], FP8, kind="ExternalOutput").ap()
        taps["p80"] = nc.dram_tensor("dbg_p80", [128, 2, 1024], I8, kind="ExternalOutput").ap()
        taps["o00"] = nc.dram_tensor("dbg_o00", [128, 264], F32, kind="ExternalOutput").ap()
        taps["xtT"] = nc.dram_tensor("dbg_xtT", [2, 128, 4, 512], BF16, kind="ExternalOutput").ap()
        taps["xt16"] = nc.dram_tensor("dbg_xt16", [4, 128, LQ], BF16, kind="ExternalOutput").ap()
        taps["xres"] = nc.dram_tensor("dbg_xres", [C, LQ], F32, kind="ExternalOutput").ap()

    with tile.TileContext(nc) as tc, ExitStack() as top:
        persist = top.enter_context(tc.tile_pool(name="persist", bufs=1))
        # PSUM during attention: 6 banks scores (3-deep, decouples the
        # exp->scores->exp dependency cycle) + 2 banks O^T.  The out-proj/LN
        # pool opens after these close (post phase).
        att_scope = ExitStack()
        ps_s = att_scope.enter_context(tc.tile_pool(name="ps_s", bufs=3, space="PSUM"))
        ps_o = att_scope.enter_context(tc.tile_pool(name="ps_o", bufs=1, space="PSUM"))
        ptp = top.enter_context(tc.tile_pool(name="ptp", bufs=1))
        nrm = top.enter_context(tc.tile_pool(name="nrm", bufs=2))
        tmp = top.enter_context(tc.tile_pool(name="tmp", bufs=2))
        yout = top.enter_context(tc.tile_pool(name="yout", bufs=4))
        ph1 = top.enter_context(tc.tile_pool(name="ph1", bufs=1))

        def alloc(shape, dt, tag):
            return persist.tile(shape, dt, tag=tag, name=tag)

        # persistent tensors
        pet16 = [alloc([128, LQ], BF16, f"pet16_{i}") for i in range(NC4)]
        wp16 = [alloc([128, C], BF16, f"wp16_{i}") for i in range(NC4)]
        gamma = [alloc([128, 1], F32, f"g_{i}") for i in range(NC4)]
        beta = [alloc([128, 1], F32, f"b_{i}") for i in range(NC4)]
        bp = [alloc([128, 1], F32, f"bp_{i}") for i in range(NC4)]

        qt8 = [alloc([128, LQ], FP8, f"qt8_{i}") for i in range(NH // 2)]
        kt8 = [alloc([128, L], FP8, f"kt8_{i}") for i in range(NH // 2)]
        # 66 cols per head: 64 dims + ones (denominator) + zero pad (dual-fp8
        # ldweights requires an even stationary free dim per k-tile)
        v8 = [alloc([128, 2, 528], FP8, f"v8_{i}") for i in range(NDM)]
        # xtT[lc]: [q 128, qb 4, (h d) 512] bf16 — normalized attention out,
        # query-major.  xt16: [hd 128, q LQ] bf16 — transposed back for the
        # output projection.
        xtT = [alloc([128, NQB, 512], BF16, f"xtT_{lc}") for lc in range(NLQ)]
        xt16 = [alloc([128, LQ], BF16, f"xt16_{i}") for i in range(NC4)]
        xres = [alloc([128, LQ], F32R, f"xr_{i}") for i in range(NC4)]

        # constants
        ones_f = alloc([128, 128], F32, "ones_f")
        onesr = alloc([128, 128], F32R, "onesr")
        nc.vector.memset(ones_f[:], 1.0)
        nc.vector.tensor_copy(onesr[:], ones_f[:])
        bias_t = alloc([128, 1], F32, "bias_t")
        nc.vector.memset(bias_t[:], BIAS_ACT)
        eps_t = alloc([128, 1], F32, "eps_t")
        nc.vector.memset(eps_t[:], LN_EPS)

        # ---- input DMAs (need-order) ---------------------------------------
        def p1load(ap_dram, shape, dt, tag):
            t = ph1.tile(shape, dt, tag=tag, name=tag)
            nc.sync.dma_start(t[:], ap_dram)
            return t
        wq8 = [p1load(wq8_d[t], [128, 2, C], FP8, f"wq8_{t}") for t in range(2)]
        pet8 = [p1load(pet8_d[t], [128, 2, LQ], FP8, f"pet8_{t}") for t in range(2)]
        wk8 = [p1load(wk8_d[t], [128, 2, C], FP8, f"wk8_{t}") for t in range(2)]
        # ct8 loaded in kcw-granular chunks so the first K projection (and
        # thus the first scores matmul) starts ~2us in instead of waiting for
        # the full 2MB
        ct8 = [ph1.tile([128, 2, L], FP8, tag=f"ct8_{t}", name=f"ct8_{t}")
               for t in range(2)]
        for kcw in range(L // 1024):
            for t in range(2):
                nc.sync.dma_start(ct8[t][:, :, kcw * 1024:(kcw + 1) * 1024],
                                  ct8_d[t][:, :, kcw * 1024:(kcw + 1) * 1024])
            if kcw == 0:
                wv8 = [p1load(wv8_d[t], [128, 2, C], FP8, f"wv8_{t}")
                       for t in range(2)]
        for i in range(NC4):
            nc.sync.dma_start(wp16[i][:], wp16_d[i])
            nc.sync.dma_start(pet16[i][:], pet16_d[i * 128:(i + 1) * 128, :])
        # tiny per-row params: one DMA each across all four chunks (HWDGE
        # slots are 625ns apiece)
        for src, dst in ((gamma_d, gamma), (beta_d, beta), (bp_d, bp)):
            for i in range(NC4):
                nc.sync.dma_start(dst[i][:], src[i * 128:(i + 1) * 128, :])

        # ---- projections (phase 1, mostly deferred into tasks) -------------
        def sps():
            return ps_s.tile([128, 1024], F32, tag="sAB", name="sp")

        def qproj(pair):
            ps = sps()
            for h in range(2):
                for t in range(2):
                    nc.tensor.matmul(
                        ps[:, h * 512:(h + 1) * 512],
                        wq8[t][:, :, pair * 128:(pair + 1) * 128],
                        pet8[t][:, :, h * 512:(h + 1) * 512],
                        start=(t == 0), stop=(t == 1), perf_mode=DR)
            e_copy(qt8[pair][:], ps[:], 1024)

        def kproj(pair, kcw):
            ps = sps()
            for h in range(2):
                sl = slice(kcw * 1024 + h * 512, kcw * 1024 + (h + 1) * 512)
                for t in range(2):
                    nc.tensor.matmul(
                        ps[:, h * 512:(h + 1) * 512],
                        wk8[t][:, :, pair * 128:(pair + 1) * 128],
                        ct8[t][:, :, sl],
                        start=(t == 0), stop=(t == 1), perf_mode=DR)
            e_copy(kt8[pair][:, kcw * 1024:(kcw + 1) * 1024], ps[:], 1024)

        def vproj(dm):
            ps = sps()
            for j in range(2):
                m = 2 * dm + j
                for t in range(2):
                    nc.tensor.matmul(
                        ps[:, j * 512:(j + 1) * 512],
                        ct8[t][:, :, m * 128:(m + 1) * 128], wv8[t][:],
                        start=(t == 0), stop=(t == 1), perf_mode=DR)
            dst = v8[dm][:].rearrange("p two (h d) -> p two h d", h=NH)[:, :, :, 0:HD]
            src = ps[:].rearrange("p (two h d) -> p two h d", two=2, h=NH)
            e_copy(dst, src, 1024)
            vv = v8[dm][:].rearrange("p two (h d) -> p two h d", h=NH)
            nc.gpsimd.tensor_copy(vv[:, :, :, HD:HD + 1], ones_f[:, 0:1]
                                  .unsqueeze(1).unsqueeze(1)
                                  .broadcast_to((128, 2, NH, 1)))
            bal.busy["gps"] += 150.0
            nc.gpsimd.memset(vv[:, :, :, HD + 1:HD + 2], 0.0)
            bal.busy["gps"] += 130.0

        # ---- task queues ---------------------------------------------------
        # urgent: must be emitted early next call (divides freeing O^T PSUM,
        #   before the next pair's first PV matmuls are emitted at dm1-j1).
        # tasks: compute-only phase-1 work, popped once per j (no stall risk).
        urgent = []
        tasks = []
        tasks2 = []   # PSUM-borrowing post work for lc0, paced 1 per dm

        def pop_task(slot):
            # slot: (dm, j)
            dm, j = slot
            if urgent and dm == 2 and j == 0:
                while urgent:
                    urgent.pop(0)()
                return
            if tasks:
                tasks.pop(0)()
                return
            if tasks2 and j == 0:
                tasks2.pop(0)()

        # eager work: only what attention(pair0, lc0)'s first dms need;
        # scores(dm) consumes kt8 kcw-window dm//4, PV(dm) consumes v8[dm]
        # one pop-slot after its vproj task fires.
        qproj(0)
        kproj(0, 0)
        tasks.append(lambda: vproj(0))
        tasks.append(lambda: kproj(0, 1))
        for dm in range(1, 4):
            tasks.append(lambda dm=dm: vproj(dm))
        tasks.append(lambda: kproj(0, 2))
        tasks.append(lambda: kproj(0, 3))

        # keep V ahead of PV consumption, then later pairs
        for dm in range(4, 8):
            tasks.append(lambda dm=dm: vproj(dm))
        for kcw in range(L // 1024):
            tasks.append(lambda kcw=kcw: kproj(1, kcw))
        tasks.append(lambda: qproj(1))
        for dm in range(8, 12):
            tasks.append(lambda dm=dm: vproj(dm))
        for kcw in range(L // 1024):
            tasks.append(lambda kcw=kcw: kproj(2, kcw))
        tasks.append(lambda: qproj(2))
        for dm in range(12, 16):
            tasks.append(lambda dm=dm: vproj(dm))
        for kcw in range(L // 1024):
            tasks.append(lambda kcw=kcw: kproj(3, kcw))
        tasks.append(lambda: qproj(3))

        # ---- attention -----------------------------------------------------
        def attention(pair, lc):
            hA, hB = 2 * pair, 2 * pair + 1
            oA = ps_o.tile([128, NQB * 66], F32, tag="oA", name="oA")
            oB = ps_o.tile([128, NQB * 66], F32, tag="oB", name="oB")

            def emit_pv(dm, p8t):
                # PSUM accumulation groups are tracked per 2KB bank (the
                # "zero region"): only the first matmul into each O^T tile
                # starts the group (lazily zeroing the bank) and only the
                # last one stops it; the qb=1..3 dm=0 matmuls overwrite
                # their still-pending-zero byte ranges.
                for h, o in ((hA, oA), (hB, oB)):
                    hl = (h % 2) * 512
                    for qb in range(NQB):
                        nc.tensor.matmul(
                            o[:, qb * 66:qb * 66 + 66],
                            p8t[:, :, hl + qb * 128:hl + (qb + 1) * 128],
                            v8[dm][:, :, h * 66:h * 66 + 66],
                            start=(dm == 0 and qb == 0),
                            stop=(dm == NDM - 1 and qb == NQB - 1),
                            perf_mode=DR)

            pend = None   # (dm, p8t): PV delayed one dm so PE never waits exp
            for dm in range(NDM):
                p8t = ptp.tile([128, 2, 1024], FP8, tag="p8", bufs=8, name="p8")
                for j in range(2):
                    m = 2 * dm + j
                    sAB = sps()
                    for h, base in ((0, 0), (1, 64)):
                        nc.tensor.matmul(
                            sAB[:, h * 512:(h + 1) * 512],
                            dr0(kt8[pair][base:base + 64, m * 128:(m + 1) * 128]),
                            dr0(qt8[pair][base:base + 64, lc * 512:(lc + 1) * 512]),
                            perf_mode=DR)
                    which = bal.pick(1024)
                    dst = p8t[:, j, :]
                    if which == "act":
                        nc.scalar.activation(
                            dst, sAB[:], mybir.ActivationFunctionType.Exp,
                            scale=SC_ACT, bias=bias_t[:])
                    else:
                        eng[which].tensor_scalar(
                            dst.bitcast(I8), sAB[:], B8EFF, 0.0,
                            AluOpType.add, AluOpType.max)
                    if pend is not None and j == 1:
                        emit_pv(*pend)
                        pend = None
                    pop_task((dm, j))
                if debug_taps and (pair, lc, dm) == (0, 0, 0):
                    nc.sync.dma_start(taps["p80"], p8t[:].bitcast(I8))
                pend = (dm, p8t)
            emit_pv(*pend)

            # normalize: xtT[q, (h d)] = O^T[q, d] / den[q]  (den = col 64).
            # The DVE TT ALU has no divide and hw allows only one non-scalar
            # PSUM input, so: reciprocal PSUM->SBUF (tiny), then TT mult with
            # a stride-0 broadcast of the reciprocal row.  Deferred (urgent
            # queue) so the next call's exps cover the last PV's latency.
            def divide(h, o):
                ov = o[:].rearrange("p (qb c) -> p qb c", qb=NQB)
                den = nrm.tile([128, NQB], F32, tag="den", bufs=8, name="den")
                bal.pick(NQB, force="dve")
                nc.vector.reciprocal(den[:], ov[:, :, HD:HD + 1])
                # normalize either as one DVE TT (broadcast mult) or as 4
                # per-qb ACT Identity ops with per-partition scale; pick
                # whichever engine is ahead so the call boundary never
                # stalls the busier exp stream.
                dst = xtT[lc][:, :, h * HD:(h + 1) * HD]
                bal.pick(NQB * HD, force="dve")
                nc.vector.tensor_tensor(
                    dst, ov[:, :, 0:HD],
                    den[:].unsqueeze(2).broadcast_to((128, NQB, HD)),
                    AluOpType.mult)
                if debug_taps and (pair, lc) == (0, 0) and h == hA:
                    nc.sync.dma_start(taps["o00"], o[:])

            def div_and_transpose():
                divide(hA, oA)
                divide(hB, oB)
                # the 4 transpose blocks of column-pair `pair` only need
                # this pair's divides (pure DMA, scheduler places them)
                for qb in range(NQB):
                    transpose_block(lc, qb, pair)

            urgent.append(div_and_transpose)

        # ---- post-attention per-lc work ------------------------------------
        def transpose_block(lc, qb, kc):
            nc.sync.dma_start(
                xt16[kc][:, lc * 512 + qb * 128:lc * 512 + (qb + 1) * 128],
                xtT[lc][:, qb, kc * 128:(kc + 1) * 128],
                transpose=True)

        def proj_chunk(lc, it, pool=None):
            sl = slice(lc * 512, (lc + 1) * 512)
            ps = (pool or pp).tile([128, 512], F32, tag="sAB" if pool else "pp",
                                   name="psy")
            for kc in range(NC4):
                nc.tensor.matmul(ps[:], wp16[kc][:, it * 128:(it + 1) * 128],
                                 xt16[kc][:, sl],
                                 start=(kc == 0), stop=(kc == NC4 - 1))
            # xres = (y + bp) + petT (reads PSUM -> DVE)
            bal.pick(512, force="dve")
            nc.vector.scalar_tensor_tensor(
                xres[it][:, sl], ps[:], bp[it][:], pet16[it][:, sl],
                AluOpType.add, AluOpType.add)

        stats = {}

        def ln_sum_chunk(lc, pool=None):
            sl = slice(lc * 512, (lc + 1) * 512)
            psum = (pool or pp).tile([1, 512], F32,
                                     tag="sAB" if pool else "pp",
                                     name="psum_sum")
            for c in range(NC4):
                nc.tensor.matmul(psum[:], onesr[:, 0:1], xres[c][:, sl],
                                 start=(c == 0), stop=(c == NC4 - 1))
            stats[("sum", lc)] = psum

        def ln_sq_chunk(lc, pool=None):
            sl = slice(lc * 512, (lc + 1) * 512)
            psq = (pool or pp).tile([1, 512], F32,
                                    tag="sAB" if pool else "pp",
                                    name="psum_sq")
            for c in range(NC4):
                xsq = tmp.tile([128, 512], F32R, tag="xsq", name="xsq")
                if pool is None:
                    # post phase: ACT is otherwise idle -- Square is in
                    # every ACT table set
                    cost = {"act": 512 * 0.8333 + 185.0,
                            "dve": 512 * 1.0417 + 60.0,
                            "gps": 512 * 0.8333 / 0.42 + 120.0}
                    e = min(("act", "dve", "gps"),
                            key=lambda k: bal.busy[k] + cost[k])
                    bal.busy[e] += cost[e]
                else:
                    e = bal.pick(512, candidates=("dve", "gps"), psum=False,
                                 tt=True)
                if e == "act":
                    nc.scalar.activation(xsq[:], xres[c][:, sl],
                                         mybir.ActivationFunctionType.Square)
                else:
                    eng[e].tensor_tensor(xsq[:], xres[c][:, sl],
                                         xres[c][:, sl], AluOpType.mult)
                nc.tensor.matmul(psq[:], onesr[:, 0:1], xsq[:],
                                 start=(c == 0), stop=(c == NC4 - 1))
            stats[("sq", lc)] = psq

        def ln_rows_chunk(lc, pool=None):
            psum = stats.pop(("sum", lc))
            psq = stats.pop(("sq", lc))
            mrow = nrm.tile([1, 512], F32R, tag=f"mu{lc}", name=f"mu{lc}")
            m2 = nrm.tile([1, 512], F32, tag=f"m2{lc}", name=f"m2{lc}")
            ve = nrm.tile([1, 512], F32, tag=f"ve{lc}", name=f"ve{lc}")
            stdr = nrm.tile([1, 512], F32R, tag=f"sd{lc}", name=f"sd{lc}")
            bal.pick(512, force="dve")
            nc.vector.tensor_scalar(mrow[:], psum[:], 1.0 / C, None,
                                    AluOpType.mult)
            bal.pick(512, force="dve", psum=False)
            nc.vector.tensor_tensor(m2[:], mrow[:], mrow[:], AluOpType.mult)
            bal.pick(512, force="dve")
            nc.vector.scalar_tensor_tensor(ve[:], psq[:], 1.0 / C, m2[:],
                                           AluOpType.mult, AluOpType.subtract)
            sdf = nrm.tile([1, 512], F32, tag=f"sf{lc}", name=f"sf{lc}")
            bal.pick(512, force="act", psum=False)
            nc.scalar.activation(sdf[:], ve[:],
                                 mybir.ActivationFunctionType.Sqrt,
                                 bias=eps_t[0:1, :])
            bal.pick(512, force="dve", psum=False)
            with nc.allow_low_precision(reason="f32r view of f32 reciprocal"):
                nc.vector.reciprocal(stdr[:], sdf[:])
            bmu = (pool or pp).tile([128, 512], F32,
                                    tag="sAB" if pool else "pp", name="bmu")
            bsd = (pool or pp).tile([128, 512], F32,
                                    tag="sAB" if pool else "pp", name="bsd")
            nc.tensor.matmul(bmu[:], onesr[0:1, :], mrow[:])
            nc.tensor.matmul(bsd[:], onesr[0:1, :], stdr[:])
            # stage broadcasts to SBUF so GPS can run the apply ops
            smu = nrm.tile([128, 512], F32, tag="smu", name="smu")
            ssd = nrm.tile([128, 512], F32, tag="ssd", name="ssd")
            e_copy(smu[:], bmu[:], 512)
            e_copy(ssd[:], bsd[:], 512)
            stats[lc] = (smu, ssd)

        def ln_apply_chunk(lc, c):
            sl = slice(lc * 512, (lc + 1) * 512)
            smu, ssd = stats[lc]
            t = tmp.tile([128, 512], F32, tag="lnt", bufs=2, name="lnt")
            y = yout.tile([128, 512], F32, tag="y", name="yout")
            # (x - smu) * ssd on one engine (DVE or GPSIMD, balanced)
            cost = {"dve": 2 * (512 * 1.0417 + 60.0),
                    "gps": 2 * (512 * 0.8333 / 0.42 + 120.0)}
            e = min(("dve", "gps"), key=lambda k: bal.busy[k] + cost[k])
            bal.busy[e] += cost[e]
            eng[e].tensor_tensor(t[:], xres[c][:, sl], smu[:],
                                 AluOpType.subtract)
            eng[e].tensor_tensor(t[:], t[:], ssd[:], AluOpType.mult)
            # * gamma + beta: per-partition scale/bias -> any of the three
            cost3 = {"act": 512 * 0.8333 + 185.0,
                     "dve": 512 * 1.0417 + 60.0,
                     "gps": 512 * 1.3889 + 120.0}
            e3 = min(("act", "dve", "gps"),
                     key=lambda k: bal.busy[k] + cost3[k])
            bal.busy[e3] += cost3[e3]
            if e3 == "act":
                nc.scalar.activation(y[:], t[:],
                                     mybir.ActivationFunctionType.Identity,
                                     bias=beta[c][:], scale=gamma[c][:])
            else:
                eng[e3].tensor_scalar(y[:], t[:], gamma[c][:], beta[c][:],
                                      AluOpType.mult, AluOpType.add)
            nc.sync.dma_start(out_d[c * 128:(c + 1) * 128, sl], y[:])

        for pair in range(NH // 2):
            attention(pair, 0)
        attention(0, 1)
        # lc0's entire post pipeline rides inside lc1's remaining attention
        # calls, borrowing one scores-pool PSUM slot at a time (costs a bit
        # of exp buffering, saves a serial tail chunk).  lc0's transposes
        # finished during the previous call, so nothing here stalls.
        for it in range(NC4):
            tasks2.append(lambda it=it: proj_chunk(0, it, ps_s))
        tasks2.append(lambda: ln_sum_chunk(0, ps_s))
        tasks2.append(lambda: ln_sq_chunk(0, ps_s))
        tasks2.append(lambda: ln_rows_chunk(0, ps_s))
        for c in range(NC4):
            tasks2.append(lambda c=c: ln_apply_chunk(0, c))
        for pair in range(1, NH // 2):
            attention(pair, 1)
        while urgent:
            urgent.pop(0)()
        while tasks:
            tasks.pop(0)()
        while tasks2:
            tasks2.pop(0)()

        # ---- post phase: out-projection + LayerNorm ------------------------
        # The attention PSUM pools close here; the post pool takes over their
        # banks.  The tile scheduler orders by readiness, so these chunks
        # pipeline across engines (PE proj -> DVE residual -> stats -> apply).
        att_scope.close()
        pp = top.enter_context(tc.tile_pool(name="pp", bufs=4, space="PSUM"))
        bal.busy = {"act": 0.0, "dve": 0.0, "gps": 0.0}
        for it in range(NC4):
            proj_chunk(1, it)
        ln_sum_chunk(1)
        ln_sq_chunk(1)
        ln_rows_chunk(1)
        for c in range(NC4):
            ln_apply_chunk(1, c)

        if debug_taps:
            for i in range(NH // 2):
                nc.sync.dma_start(taps["qt"][i * 128:(i + 1) * 128, :], qt8[i][:])
                nc.sync.dma_start(taps["kt"][i * 128:(i + 1) * 128, :], kt8[i][:])
            for dm in range(NDM):
                nc.sync.dma_start(taps["vt"][dm], v8[dm][:])
            for lc in range(NLQ):
                nc.sync.dma_start(taps["xtT"][lc], xtT[lc][:])
            for i in range(NC4):
                nc.sync.dma_start(taps["xt16"][i], xt16[i][:])
                nc.sync.dma_start(taps["xres"][i * 128:(i + 1) * 128, :],
                                  xres[i][:].bitcast(F32))

    nc.compile()
    return nc


def prep_core_inputs(inputs):
    """Shard + lay out the full inputs for the 8 cores."""
    pet = np.asarray(inputs["pet_feat"], np.float32).reshape(B, C, L)
    ct = np.asarray(inputs["ct_feat"], np.float32).reshape(B, C, L)
    bf = ml_dtypes.bfloat16

    def wprep(w):
        # [2(t), 128(p), 2(j), 512(out)]: value = W[out, 256t+128j+p]
        wt = np.ascontiguousarray(np.asarray(w, np.float32).T)  # [in, out]
        return np.ascontiguousarray(
            wt.reshape(2, 2, 128, C).transpose(0, 2, 1, 3)).astype(E4)

    wq8 = wprep(inputs["Wq"])
    wk8 = wprep(inputs["Wk"])
    wv8 = wprep(inputs["Wv"])
    # wp16[kc][p][out] = Wp[out, kc*128+p]
    wpT = np.asarray(inputs["Wp"], np.float32).T
    wp16 = np.ascontiguousarray(wpT.reshape(NC4, 128, C)).astype(bf)
    gamma = np.asarray(inputs["gamma"], np.float32).reshape(C, 1)
    beta = np.asarray(inputs["beta"], np.float32).reshape(C, 1)
    bp = np.asarray(inputs["bp"], np.float32).reshape(C, 1)

    ct8 = {}
    for b in range(B):
        ct8[b] = np.ascontiguousarray(
            ct[b].reshape(2, 2, 128, L).transpose(0, 2, 1, 3)).astype(E4)

    in_maps = []
    for core in range(NCORES):
        b, jq = divmod(core, 4)
        sl = slice(jq * LQ, (jq + 1) * LQ)
        pet_sl = np.ascontiguousarray(pet[b][:, sl])
        pet8 = np.ascontiguousarray(
            (pet_sl * QPRESCALE).reshape(2, 2, 128, LQ).transpose(0, 2, 1, 3)
        ).astype(E4)
        in_maps.append({
            "pet8": pet8,
            "ct8": ct8[b],
            "pet16": pet_sl.astype(bf),
            "wq8": wq8, "wk8": wk8, "wv8": wv8, "wp16": wp16,
            "gamma": gamma, "beta": beta, "bp": bp,
        })
    return in_maps


def assemble_output(results):
    out = np.empty((B, C, L), np.float32)
    for core in range(NCORES):
        b, jq = divmod(core, 4)
        out[b][:, jq * LQ:(jq + 1) * LQ] = results[core]["out"]
    return out.reshape(B, C, H, W)


_NC_CACHE = {}


def get_nc(debug=False, debug_taps=False):
    key = (debug, debug_taps)
    if key not in _NC_CACHE:
        _NC_CACHE[key] = build_nc(debug=debug, debug_taps=debug_taps)
    return _NC_CACHE[key]


def kernel(**inputs):
    nc = get_nc()
    in_maps = prep_core_inputs(inputs)
    res = bass_utils.run_bass_kernel_spmd(nc, in_maps, list(range(NCORES)))
    return assemble_output(res.results)


# revision 51
# speedup vs baseline: 1.4377x; 1.0003x over previous
# Trainium2 Bass kernel for CrossAttentionFusion — fp8 DoubleRow + transposed
# PV rewrite.
#
# Reference computation (per batch b):
#   pet_seq = pet_feat[b] viewed as (C, L)^T            L = H*W = 4096, C = 512
#   q = pet_seq @ Wq.T ; k = ct_seq @ Wk.T ; v = ct_seq @ Wv.T   (8 heads, hd=64)
#   x = softmax(q k^T / sqrt(hd)) v                      per head
#   y = LN(pet_seq + x @ Wp.T + bp) * gamma + beta       -> (B, C, H, W)
#
# Sharding: 8 cores = 2 batches x 4 query-row chunks (1024 rows each), no
# collectives.
#
# Design notes (v2 — tuned against the TimelineSim cost model; 310us -> 226.7us):
#   * The kernel is elementwise-bound: the softmax exp must read f32 PSUM and
#     write fp8 SBUF, which only ACT/DVE can do (GPSIMD has no PSUM port and
#     DMA cannot touch PSUM — both verified against the walrus verifier).
#     All structural choices minimize non-exp ACT/DVE work and keep both
#     engines saturated (~90% busy through the attention phase).
#   * Scores: fp8e4 DoubleRow with a stride-0 broadcast contraction (doubles
#     the product; q side pre-scaled by A8*scale/2 so PSUM = A8*logit).
#   * exp runs split across ACT (table exp) and DVE (Schraudolph int8 bit
#     trick) chosen per-op by a cost-model greedy balancer.  The scores PSUM
#     pool is 3 tiles deep: with only 2, the exp->scores->exp dependency
#     cycle (~1.7us) caps exp issue at ~62% of engine capacity.
#   * PV runs TRANSPOSED: P8 is the matmul stationary ([128k, 2, 128q]
#     slices), V the moving operand, so O^T lands as [128 q-part, 4qb x 66]
#     with the softmax denominator (ones column in V) at column 64.  PSUM
#     accumulation groups are bank-granular: only the first matmul into each
#     O^T tile sets start (lazy-zeroing the bank), only the last sets stop.
#     Normalization is then per-PARTITION: one tiny reciprocal plus one
#     broadcast tensor_tensor mult per head (the DVE TT ALU has no divide,
#     and only one non-scalar PSUM input is allowed per instruction) — no
#     denominator DMAs or PE broadcasts.  Divides are deferred to the next
#     call's dm1 slot (urgent queue) so the busier exp stream covers the
#     last PV matmul's latency; they must be emitted before the next pair's
#     first PV (PV trails the exp stream by two dm) because the O^T pool is
#     1-deep.
#   * xt is produced in bf16 [q, hd] layout and flipped back to [hd, q] with
#     32 XBAR DMA transposes (14 ns/16x128 tile, on the otherwise idle DMA
#     path), emitted right after each pair's divides.  Out-projection is a
#     plain bf16 matmul (PE has large slack).
#   * Phase-1 (Q/K/V projections + fp8 copies) shares the scores PSUM pool
#     and is drip-fed into the attention stream as micro-tasks (one per j)
#     so ACT/DVE never drain; weight/pet loads are single DMAs spanning both
#     contraction halves (HWDGE charges ~625ns per DMA instruction) and ct8
#     is kcw-chunked, so the first scores start ~2us in.
#   * lc0's out-proj/LN pipeline runs inside lc1's attention calls,
#     borrowing one scores-pool PSUM slot at a time; lc1's runs as a post
#     phase whose PSUM pool reuses the attention banks (pool aliasing blocks
#     any earlier start).  LN: channel sums via ones-column f32r matmuls, row stats
#     on [1,512] strips (Sqrt on ACT — Exp and Sqrt live in different ACT
#     table sets, so the one unavoidable 1.3us table swap happens here),
#     mean/istd broadcast via K=1 matmuls, apply split [sub+mult on DVE or
#     GPSIMD, gamma/beta as per-partition scale+bias on whichever of
#     ACT/DVE/GPSIMD is ahead].

import numpy as np
import ml_dtypes

import concourse.bacc as bacc
import concourse.bass as bass
import concourse.tile as tile
from concourse import mybir
from concourse import bass_utils
from concourse.alu_op_type import AluOpType
from contextlib import ExitStack

F32 = mybir.dt.float32
F32R = mybir.dt.float32r
BF16 = mybir.dt.bfloat16
FP8 = mybir.dt.float8e4
I8 = mybir.dt.int8
E4 = ml_dtypes.float8_e4m3
DR = mybir.MatmulPerfMode.DoubleRow

B, C, H, W = 2, 512, 64, 64
L = H * W                    # 4096
NH, HD = 8, 64
NCORES = 8
LQ = L // 4                  # 1024 query rows per core
ATT_SCALE = HD ** -0.5       # 1/8
LN_EPS = 1e-5

# fp8 Schraudolph constants (e4m3: 4 exp bits bias 7, 3 mantissa bits).
A8 = 8.0 / np.log(2.0)            # octave slope in bit units
B8 = 56.05                        # 7*8 + truncation/Schraudolph tuning
B8EFF = 32.0                      # bias added before int8 trunc
SH = (B8 - B8EFF) / A8            # effective logit shift (~2.084)
QPRESCALE = float(A8 * ATT_SCALE / 2.0)   # pet8 host prescale (DR doubles)
SC_ACT = float(1.0 / A8)
BIAS_ACT = float(-SH)

NC4 = C // 128           # 4 channel chunks of 128
NM = L // 128            # 32 key m-chunks
NDM = NM // 2            # 16 double m-chunks
NLQ = LQ // 512          # 2 query chunks per core
NQB = 4                  # 128-query blocks per lc chunk


def dr0(ap):
    """Stride-0 DoubleRow plane dim: [K, N] -> [K, 2, N] reading data twice."""
    return ap.unsqueeze(1).broadcast_to((ap.shape[0], 2, ap.shape[1]))


class EngineBalancer:
    """Greedy engine assignment using the v2 instruction-cost model.

    engine-busy ns = slope*cols + init/2 where init = 2*access_cycles of the
    slowest operand space.  GPSIMD adds the Q7 launch and runs TT at 0.42
    efficiency.  GPSIMD cannot access PSUM.
    """
    SLOPE = {"act": 0.8333, "dve": 1.0417, "gps": 1.3889}
    INIT_PSUM = {"act": 143.0, "dve": 125.0}
    INIT_SBUF = {"act": 185.0, "dve": 60.0, "gps": 120.0}

    def __init__(self):
        self.busy = {"act": 0.0, "dve": 0.0, "gps": 0.0}

    def cost(self, e, cols, psum=True, tt=False):
        c = self.SLOPE[e] * cols
        if e == "gps":
            if tt:
                c = cols * 0.8333 / 0.42
            return c + self.INIT_SBUF["gps"]
        init = self.INIT_PSUM[e] if psum else self.INIT_SBUF[e]
        return c + init

    def pick(self, cols, force=None, candidates=("act", "dve"), psum=True,
             tt=False):
        if force is None:
            e = min(candidates,
                    key=lambda k: self.busy[k] + self.cost(k, cols, psum, tt))
        else:
            e = force
        self.busy[e] += self.cost(e, cols, psum, tt)
        return e


def build_nc(debug: bool = False, debug_taps: bool = False):
    nc = bacc.Bacc("TRN2", target_bir_lowering=False, debug=debug,
                   num_devices=NCORES)
    eng = {"act": nc.scalar, "dve": nc.vector, "gps": nc.gpsimd}
    bal = EngineBalancer()

    def e_copy(dst, src, cols, force=None, psum=True):
        which = bal.pick(cols, force, psum=psum)
        if which == "act":
            nc.scalar.copy(dst, src)
        else:
            eng[which].tensor_copy(dst, src)

    # ---- DRAM I/O ----------------------------------------------------------
    pet8_d = nc.dram_tensor("pet8", [2, 128, 2, LQ], FP8, kind="ExternalInput").ap()
    ct8_d = nc.dram_tensor("ct8", [2, 128, 2, L], FP8, kind="ExternalInput").ap()
    pet16_d = nc.dram_tensor("pet16", [C, LQ], BF16, kind="ExternalInput").ap()
    wq8_d = nc.dram_tensor("wq8", [2, 128, 2, C], FP8, kind="ExternalInput").ap()
    wk8_d = nc.dram_tensor("wk8", [2, 128, 2, C], FP8, kind="ExternalInput").ap()
    wv8_d = nc.dram_tensor("wv8", [2, 128, 2, C], FP8, kind="ExternalInput").ap()
    wp16_d = nc.dram_tensor("wp16", [NC4, 128, C], BF16, kind="ExternalInput").ap()
    gamma_d = nc.dram_tensor("gamma", [C, 1], F32, kind="ExternalInput").ap()
    beta_d = nc.dram_tensor("beta", [C, 1], F32, kind="ExternalInput").ap()
    bp_d = nc.dram_tensor("bp", [C, 1], F32, kind="ExternalInput").ap()
    out_d = nc.dram_tensor("out", [C, LQ], F32, kind="ExternalOutput").ap()
    taps = {}
    if debug_taps:
        taps["qt"] = nc.dram_tensor("dbg_qt", [C, LQ], FP8, kind="ExternalOutput").ap()
        taps["kt"] = nc.dram_tensor("dbg_kt", [C, L], FP8, kind="ExternalOutput").ap()
        taps["vt"] = nc.dram_tensor("dbg_vt", [16, 128, 2, 528], FP8, kind="ExternalOutput").ap()
        taps["p80"] = nc.dram_tensor("dbg_p80", [128, 2, 1024], I8, kind="ExternalOutput").ap()
        taps["o00"] = nc.dram_tensor("dbg_o00", [128, 264], F32, kind="ExternalOutput").ap()
        taps["xtT"] = nc.dram_tensor("dbg_xtT", [2, 128, 4, 512], BF16, kind="ExternalOutput").ap()
        taps["xt16"] = nc.dram_tensor("dbg_xt16", [4, 128, LQ], BF16, kind="ExternalOutput").ap()
        taps["xres"] = nc.dram_tensor("dbg_xres", [C, LQ], F32, kind="ExternalOutput").ap()

    with tile.TileContext(nc) as tc, ExitStack() as top:
        persist = top.enter_context(tc.tile_pool(name="persist", bufs=1))
        # PSUM during attention: 6 banks scores (3-deep, decouples the
        # exp->scores->exp dependency cycle) + 2 banks O^T.  The out-proj/LN
        # pool opens after these close (post phase).
        att_scope = ExitStack()
        ps_s = att_scope.enter_context(tc.tile_pool(name="ps_s", bufs=3, space="PSUM"))
        ps_o = att_scope.enter_context(tc.tile_pool(name="ps_o", bufs=1, space="PSUM"))
        ptp = top.enter_context(tc.tile_pool(name="ptp", bufs=1))
        nrm = top.enter_context(tc.tile_pool(name="nrm", bufs=2))
        tmp = top.enter_context(tc.tile_pool(name="tmp", bufs=2))
        yout = top.enter_context(tc.tile_pool(name="yout", bufs=4))
        ph1 = top.enter_context(tc.tile_pool(name="ph1", bufs=1))

        def alloc(shape, dt, tag):
            return persist.tile(shape, dt, tag=tag, name=tag)

        # persistent tensors
        pet16 = [alloc([128, LQ], BF16, f"pet16_{i}") for i in range(NC4)]
        wp16 = [alloc([128, C], BF16, f"wp16_{i}") for i in range(NC4)]
        gamma = [alloc([128, 1], F32, f"g_{i}") for i in range(NC4)]
        beta = [alloc([128, 1], F32, f"b_{i}") for i in range(NC4)]
        bp = [alloc([128, 1], F32, f"bp_{i}") for i in range(NC4)]

        qt8 = [alloc([128, LQ], FP8, f"qt8_{i}") for i in range(NH // 2)]
        kt8 = [alloc([128, L], FP8, f"kt8_{i}") for i in range(NH // 2)]
        # 66 cols per head: 64 dims + ones (denominator) + zero pad (dual-fp8
        # ldweights requires an even stationary free dim per k-tile)
        v8 = [alloc([128, 2, 528], FP8, f"v8_{i}") for i in range(NDM)]
        # xtT[lc]: [q 128, qb 4, (h d) 512] bf16 — normalized attention out,
        # query-major.  xt16: [hd 128, q LQ] bf16 — transposed back for the
        # output projection.
        xtT = [alloc([128, NQB, 512], BF16, f"xtT_{lc}") for lc in range(NLQ)]
        xt16 = [alloc([128, LQ], BF16, f"xt16_{i}") for i in range(NC4)]
        xres = [alloc([128, LQ], F32R, f"xr_{i}") for i in range(NC4)]

        # constants
        ones_f = alloc([128, 128], F32, "ones_f")
        onesr = alloc([128, 128], F32R, "onesr")
        nc.vector.memset(ones_f[:], 1.0)
        nc.vector.tensor_copy(onesr[:], ones_f[:])
        bias_t = alloc([128, 1], F32, "bias_t")
        nc.vector.memset(bias_t[:], BIAS_ACT)
        eps_t = alloc([128, 1], F32, "eps_t")
        nc.vector.memset(eps_t[:], LN_EPS)

        # ---- input DMAs (need-order) ---------------------------------------
        def p1load(ap_dram, shape, dt, tag):
            t = ph1.tile(shape, dt, tag=tag, name=tag)
            nc.sync.dma_start(t[:], ap_dram)
            return t
        wq8 = [p1load(wq8_d[t], [128, 2, C], FP8, f"wq8_{t}") for t in range(2)]
        pet8 = [p1load(pet8_d[t], [128, 2, LQ], FP8, f"pet8_{t}") for t in range(2)]
        wk8 = [p1load(wk8_d[t], [128, 2, C], FP8, f"wk8_{t}") for t in range(2)]
        # ct8 loaded in kcw-granular chunks so the first K projection (and
        # thus the first scores matmul) starts ~2us in instead of waiting for
        # the full 2MB
        ct8 = [ph1.tile([128, 2, L], FP8, tag=f"ct8_{t}", name=f"ct8_{t}")
               for t in range(2)]
        for kcw in range(L // 1024):
            for t in range(2):
                nc.sync.dma_start(ct8[t][:, :, kcw * 1024:(kcw + 1) * 1024],
                                  ct8_d[t][:, :, kcw * 1024:(kcw + 1) * 1024])
            if kcw == 0:
                wv8 = [p1load(wv8_d[t], [128, 2, C], FP8, f"wv8_{t}")
                       for t in range(2)]
        for i in range(NC4):
            nc.sync.dma_start(wp16[i][:], wp16_d[i])
            nc.sync.dma_start(pet16[i][:], pet16_d[i * 128:(i + 1) * 128, :])
        # tiny per-row params: one DMA each across all four chunks (HWDGE
        # slots are 625ns apiece)
        for src, dst in ((gamma_d, gamma), (beta_d, beta), (bp_d, bp)):
            for i in range(NC4):
                nc.sync.dma_start(dst[i][:], src[i * 128:(i + 1) * 128, :])

        # ---- projections (phase 1, mostly deferred into tasks) -------------
        def sps():
            return ps_s.tile([128, 1024], F32, tag="sAB", name="sp")

        def qproj(pair):
            ps = sps()
            for h in range(2):
                for t in range(2):
                    nc.tensor.matmul(
                        ps[:, h * 512:(h + 1) * 512],
                        wq8[t][:, :, pair * 128:(pair + 1) * 128],
                        pet8[t][:, :, h * 512:(h + 1) * 512],
                        start=(t == 0), stop=(t == 1), perf_mode=DR)
            e_copy(qt8[pair][:], ps[:], 1024)

        def kproj(pair, kcw):
            ps = sps()
            for h in range(2):
                sl = slice(kcw * 1024 + h * 512, kcw * 1024 + (h + 1) * 512)
                for t in range(2):
                    nc.tensor.matmul(
                        ps[:, h * 512:(h + 1) * 512],
                        wk8[t][:, :, pair * 128:(pair + 1) * 128],
                        ct8[t][:, :, sl],
                        start=(t == 0), stop=(t == 1), perf_mode=DR)
            e_copy(kt8[pair][:, kcw * 1024:(kcw + 1) * 1024], ps[:], 1024)

        def vproj(dm):
            ps = sps()
            for j in range(2):
                m = 2 * dm + j
                for t in range(2):
                    nc.tensor.matmul(
                        ps[:, j * 512:(j + 1) * 512],
                        ct8[t][:, :, m * 128:(m + 1) * 128], wv8[t][:],
                        start=(t == 0), stop=(t == 1), perf_mode=DR)
            dst = v8[dm][:].rearrange("p two (h d) -> p two h d", h=NH)[:, :, :, 0:HD]
            src = ps[:].rearrange("p (two h d) -> p two h d", two=2, h=NH)
            e_copy(dst, src, 1024)
            vv = v8[dm][:].rearrange("p two (h d) -> p two h d", h=NH)
            nc.gpsimd.tensor_copy(vv[:, :, :, HD:HD + 1], ones_f[:, 0:1]
                                  .unsqueeze(1).unsqueeze(1)
                                  .broadcast_to((128, 2, NH, 1)))
            bal.busy["gps"] += 150.0
            nc.gpsimd.memset(vv[:, :, :, HD + 1:HD + 2], 0.0)
            bal.busy["gps"] += 130.0

        # ---- task queues ---------------------------------------------------
        # urgent: must be emitted early next call (divides freeing O^T PSUM,
        #   before the next pair's first PV matmuls are emitted at dm1-j1).
        # tasks: compute-only phase-1 work, popped once per j (no stall risk).
        urgent = []
        tasks = []
        tasks2 = []   # PSUM-borrowing post work for lc0, paced 1 per dm

        def pop_task(slot):
            # slot: (dm, j)
            dm, j = slot
            if urgent and dm == 2 and j == 0:
                while urgent:
                    urgent.pop(0)()
                return
            if tasks:
                tasks.pop(0)()
                return
            if tasks2 and j == 0:
                tasks2.pop(0)()

        # eager work: only what attention(pair0, lc0)'s first dms need;
        # scores(dm) consumes kt8 kcw-window dm//4, PV(dm) consumes v8[dm]
        # one pop-slot after its vproj task fires.
        qproj(0)
        kproj(0, 0)
        tasks.append(lambda: vproj(0))
        tasks.append(lambda: kproj(0, 1))
        for dm in range(1, 4):
            tasks.append(lambda dm=dm: vproj(dm))
        tasks.append(lambda: kproj(0, 2))
        tasks.append(lambda: kproj(0, 3))

        # keep V ahead of PV consumption, then later pairs
        for dm in range(4, 8):
            tasks.append(lambda dm=dm: vproj(dm))
        for kcw in range(L // 1024):
            tasks.append(lambda kcw=kcw: kproj(1, kcw))
        tasks.append(lambda: qproj(1))
        for dm in range(8, 12):
            tasks.append(lambda dm=dm: vproj(dm))
        for kcw in range(L // 1024):
            tasks.append(lambda kcw=kcw: kproj(2, kcw))
        tasks.append(lambda: qproj(2))
        for dm in range(12, 16):
            tasks.append(lambda dm=dm: vproj(dm))
        for kcw in range(L // 1024):
            tasks.append(lambda kcw=kcw: kproj(3, kcw))
        tasks.append(lambda: qproj(3))

        # ---- attention -----------------------------------------------------
        def attention(pair, lc):
            hA, hB = 2 * pair, 2 * pair + 1
            oA = ps_o.tile([128, NQB * 66], F32, tag="oA", name="oA")
            oB = ps_o.tile([128, NQB * 66], F32, tag="oB", name="oB")

            def emit_pv(dm, p8t):
                # PSUM accumulation groups are tracked per 2KB bank (the
                # "zero region"): only the first matmul into each O^T tile
                # starts the group (lazily zeroing the bank) and only the
                # last one stops it; the qb=1..3 dm=0 matmuls overwrite
                # their still-pending-zero byte ranges.
                for h, o in ((hA, oA), (hB, oB)):
                    hl = (h % 2) * 512
                    for qb in range(NQB):
                        nc.tensor.matmul(
                            o[:, qb * 66:qb * 66 + 66],
                            p8t[:, :, hl + qb * 128:hl + (qb + 1) * 128],
                            v8[dm][:, :, h * 66:h * 66 + 66],
                            start=(dm == 0 and qb == 0),
                            stop=(dm == NDM - 1 and qb == NQB - 1),
                            perf_mode=DR)

            pend = None   # (dm, p8t): PV delayed one dm so PE never waits exp
            for dm in range(NDM):
                p8t = ptp.tile([128, 2, 1024], FP8, tag="p8", bufs=8, name="p8")
                for j in range(2):
                    m = 2 * dm + j
                    sAB = sps()
                    for h, base in ((0, 0), (1, 64)):
                        nc.tensor.matmul(
                            sAB[:, h * 512:(h + 1) * 512],
                            dr0(kt8[pair][base:base + 64, m * 128:(m + 1) * 128]),
                            dr0(qt8[pair][base:base + 64, lc * 512:(lc + 1) * 512]),
                            perf_mode=DR)
                    which = bal.pick(1024)
                    dst = p8t[:, j, :]
                    if which == "act":
                        nc.scalar.activation(
                            dst, sAB[:], mybir.ActivationFunctionType.Exp,
                            scale=SC_ACT, bias=bias_t[:])
                    else:
                        eng[which].tensor_scalar(
                            dst.bitcast(I8), sAB[:], B8EFF, 0.0,
                            AluOpType.add, AluOpType.max)
                    if pend is not None and j == 1:
                        emit_pv(*pend)
                        pend = None
                    pop_task((dm, j))
                if debug_taps and (pair, lc, dm) == (0, 0, 0):
                    nc.sync.dma_start(taps["p80"], p8t[:].bitcast(I8))
                pend = (dm, p8t)
            emit_pv(*pend)

            # normalize: xtT[q, (h d)] = O^T[q, d] / den[q]  (den = col 64).
            # The DVE TT ALU has no divide and hw allows only one non-scalar
            # PSUM input, so: reciprocal PSUM->SBUF (tiny), then TT mult with
            # a stride-0 broadcast of the reciprocal row.  Deferred (urgent
            # queue) so the next call's exps cover the last PV's latency.
            def divide(h, o):
                ov = o[:].rearrange("p (qb c) -> p qb c", qb=NQB)
                den = nrm.tile([128, NQB], F32, tag="den", bufs=8, name="den")
                bal.pick(NQB, force="dve")
                nc.vector.reciprocal(den[:], ov[:, :, HD:HD + 1])
                # normalize either as one DVE TT (broadcast mult) or as 4
                # per-qb ACT Identity ops with per-partition scale; pick
                # whichever engine is ahead so the call boundary never
                # stalls the busier exp stream.
                dst = xtT[lc][:, :, h * HD:(h + 1) * HD]
                bal.pick(NQB * HD, force="dve")
                nc.vector.tensor_tensor(
                    dst, ov[:, :, 0:HD],
                    den[:].unsqueeze(2).broadcast_to((128, NQB, HD)),
                    AluOpType.mult)
                if debug_taps and (pair, lc) == (0, 0) and h == hA:
                    nc.sync.dma_start(taps["o00"], o[:])

            def div_and_transpose():
                divide(hA, oA)
                divide(hB, oB)
                # the 4 transpose blocks of column-pair `pair` only need
                # this pair's divides (pure DMA, scheduler places them)
                for qb in range(NQB):
                    transpose_block(lc, qb, pair)

            urgent.append(div_and_transpose)

        # ---- post-attention per-lc work ------------------------------------
        def transpose_block(lc, qb, kc):
            nc.sync.dma_start(
                xt16[kc][:, lc * 512 + qb * 128:lc * 512 + (qb + 1) * 128],
                xtT[lc][:, qb, kc * 128:(kc + 1) * 128],
                transpose=True)

        def proj_chunk(lc, it, pool=None):
            sl = slice(lc * 512, (lc + 1) * 512)
            ps = (pool or pp).tile([128, 512], F32, tag="sAB" if pool else "pp",
                                   name="psy")
            for kc in range(NC4):
                nc.tensor.matmul(ps[:], wp16[kc][:, it * 128:(it + 1) * 128],
                                 xt16[kc][:, sl],
                                 start=(kc == 0), stop=(kc == NC4 - 1))
            # xres = (y + bp) + petT (reads PSUM -> DVE)
            bal.pick(512, force="dve")
            nc.vector.scalar_tensor_tensor(
                xres[it][:, sl], ps[:], bp[it][:], pet16[it][:, sl],
                AluOpType.add, AluOpType.add)

        stats = {}

        def ln_sum_chunk(lc, pool=None):
            sl = slice(lc * 512, (lc + 1) * 512)
            psum = (pool or pp).tile([1, 512], F32,
                                     tag="sAB" if pool else "pp",
                                     name="psum_sum")
            for c in range(NC4):
                nc.tensor.matmul(psum[:], onesr[:, 0:1], xres[c][:, sl],
                                 start=(c == 0), stop=(c == NC4 - 1))
            stats[("sum", lc)] = psum

        def ln_sq_chunk(lc, pool=None):
            sl = slice(lc * 512, (lc + 1) * 512)
            psq = (pool or pp).tile([1, 512], F32,
                                    tag="sAB" if pool else "pp",
                                    name="psum_sq")
            for c in range(NC4):
                xsq = tmp.tile([128, 512], F32R, tag="xsq", name="xsq")
                if pool is None:
                    # post phase: ACT is otherwise idle -- Square is in
                    # every ACT table set
                    cost = {"act": 512 * 0.8333 + 185.0,
                            "dve": 512 * 1.0417 + 60.0,
                            "gps": 512 * 0.8333 / 0.42 + 120.0}
                    e = min(("act", "dve", "gps"),
                            key=lambda k: bal.busy[k] + cost[k])
                    bal.busy[e] += cost[e]
                else:
                    e = bal.pick(512, candidates=("dve", "gps"), psum=False,
                                 tt=True)
                if e == "act":
                    nc.scalar.activation(xsq[:], xres[c][:, sl],
                                         mybir.ActivationFunctionType.Square)
                else:
                    eng[e].tensor_tensor(xsq[:], xres[c][:, sl],
                                         xres[c][:, sl], AluOpType.mult)
                nc.tensor.matmul(psq[:], onesr[:, 0:1], xsq[:],
                                 start=(c == 0), stop=(c == NC4 - 1))
            stats[("sq", lc)] = psq

        def ln_rows_chunk(lc, pool=None):
            psum = stats.pop(("sum", lc))
            psq = stats.pop(("sq", lc))
            mrow = nrm.tile([1, 512], F32R, tag=f"mu{lc}", name=f"mu{lc}")
            m2 = nrm.tile([1, 512], F32, tag=f"m2{lc}", name=f"m2{lc}")
            ve = nrm.tile([1, 512], F32, tag=f"ve{lc}", name=f"ve{lc}")
            stdr = nrm.tile([1, 512], F32R, tag=f"sd{lc}", name=f"sd{lc}")
            bal.pick(512, force="dve")
            nc.vector.tensor_scalar(mrow[:], psum[:], 1.0 / C, None,
                                    AluOpType.mult)
            bal.pick(512, force="dve", psum=False)
            nc.vector.tensor_tensor(m2[:], mrow[:], mrow[:], AluOpType.mult)
            bal.pick(512, force="dve")
            nc.vector.scalar_tensor_tensor(ve[:], psq[:], 1.0 / C, m2[:],
                                           AluOpType.mult, AluOpType.subtract)
            sdf = nrm.tile([1, 512], F32, tag=f"sf{lc}", name=f"sf{lc}")
            bal.pick(512, force="act", psum=False)
            nc.scalar.activation(sdf[:], ve[:],
                                 mybir.ActivationFunctionType.Sqrt,
                                 bias=eps_t[0:1, :])
            bal.pick(512, force="dve", psum=False)
            with nc.allow_low_precision(reason="f32r view of f32 reciprocal"):
                nc.vector.reciprocal(stdr[:], sdf[:])
            bmu = (pool or pp).tile([128, 512], F32,
                                    tag="sAB" if pool else "pp", name="bmu")
            bsd = (pool or pp).tile([128, 512], F32,
                                    tag="sAB" if pool else "pp", name="bsd")
            nc.tensor.matmul(bmu[:], onesr[0:1, :], mrow[:])
            nc.tensor.matmul(bsd[:], onesr[0:1, :], stdr[:])
            # stage broadcasts to SBUF so GPS can run the apply ops
            smu = nrm.tile([128, 512], F32, tag="smu", name="smu")
            ssd = nrm.tile([128, 512], F32, tag="ssd", name="ssd")
            e_copy(smu[:], bmu[:], 512)
            e_copy(ssd[:], bsd[:], 512)
            stats[lc] = (smu, ssd)

        def ln_apply_chunk(lc, c):
            sl = slice(lc * 512, (lc + 1) * 512)
            smu, ssd = stats[lc]
            t = tmp.tile([128, 512], F32, tag="lnt", bufs=2, name="lnt")
            y = yout.tile([128, 512], F32, tag="y", name="yout")
            # (x - smu) * ssd on one engine (DVE or GPSIMD, balanced)
            cost = {"dve": 2 * (512 * 1.0417 + 60.0),
                    "gps": 2 * (512 * 0.8333 / 0.42 + 120.0)}
            e = min(("dve", "gps"), key=lambda k: bal.busy[k] + cost[k])
            bal.busy[e] += cost[e]
            eng[e].tensor_tensor(t[:], xres[c][:, sl], smu[:],
                                 AluOpType.subtract)
            eng[e].tensor_tensor(t[:], t[:], ssd[:], AluOpType.mult)
            # * gamma + beta: per-partition scale/bias -> any of the three
            cost3 = {"act": 512 * 0.8333 + 185.0,
                     "dve": 512 * 1.0417 + 60.0,
                     "gps": 512 * 1.3889 + 120.0}
            e3 = min(("act", "dve", "gps"),
                     key=lambda k: bal.busy[k] + cost3[k])
            bal.busy[e3] += cost3[e3]
            if e3 == "act":
                nc.scalar.activation(y[:], t[:],
                                     mybir.ActivationFunctionType.Identity,
                                     bias=beta[c][:], scale=gamma[c][:])
            else:
                eng[e3].tensor_scalar(y[:], t[:], gamma[c][:], beta[c][:],
                                      AluOpType.mult, AluOpType.add)
            nc.sync.dma_start(out_d[c * 128:(c + 1) * 128, sl], y[:])

        for pair in range(NH // 2):
            attention(pair, 0)
        attention(0, 1)
        # lc0's entire post pipeline rides inside lc1's remaining attention
        # calls, borrowing one scores-pool PSUM slot at a time (costs a bit
        # of exp buffering, saves a serial tail chunk).  lc0's transposes
        # finished during the previous call, so nothing here stalls.
        for it in range(NC4):
            tasks2.append(lambda it=it: proj_chunk(0, it, ps_s))
        tasks2.append(lambda: ln_sum_chunk(0, ps_s))
        tasks2.append(lambda: ln_sq_chunk(0, ps_s))
        tasks2.append(lambda: ln_rows_chunk(0, ps_s))
        for c in range(NC4):
            tasks2.append(lambda c=c: ln_apply_chunk(0, c))
        for pair in range(1, NH // 2):
            attention(pair, 1)
        while urgent:
            urgent.pop(0)()
        while tasks:
            tasks.pop(0)()
        while tasks2:
            tasks2.pop(0)()

        # ---- post phase: out-projection + LayerNorm ------------------------
        # The attention PSUM pools close here; the post pool takes over their
        # banks.  The tile scheduler orders by readiness, so these chunks
        # pipeline across engines (PE proj -> DVE residual -> stats -> apply).
        att_scope.close()
        pp = top.enter_context(tc.tile_pool(name="pp", bufs=4, space="PSUM"))
        bal.busy = {"act": 0.0, "dve": 0.0, "gps": 0.0}
        for it in range(NC4):
            proj_chunk(1, it)
        ln_sum_chunk(1)
        ln_sq_chunk(1)
        ln_rows_chunk(1)
        for c in range(NC4):
            ln_apply_chunk(1, c)

        if debug_taps:
            for i in range(NH // 2):
                nc.sync.dma_start(taps["qt"][i * 128:(i + 1) * 128, :], qt8[i][:])
                nc.sync.dma_start(taps["kt"][i * 128:(i + 1) * 128, :], kt8[i][:])
            for dm in range(NDM):
                nc.sync.dma_start(taps["vt"][dm], v8[dm][:])
            for lc in range(NLQ):
                nc.sync.dma_start(taps["xtT"][lc], xtT[lc][:])
            for i in range(NC4):
                nc.sync.dma_start(taps["xt16"][i], xt16[i][:])
                nc.sync.dma_start(taps["xres"][i * 128:(i + 1) * 128, :],
                                  xres[i][:].bitcast(F32))

    nc.compile()
    return nc


def prep_core_inputs(inputs):
    """Shard + lay out the full inputs for the 8 cores."""
    pet = np.asarray(inputs["pet_feat"], np.float32).reshape(B, C, L)
    ct = np.asarray(inputs["ct_feat"], np.float32).reshape(B, C, L)
    bf = ml_dtypes.bfloat16

    def wprep(w):
        # [2(t), 128(p), 2(j), 512(out)]: value = W[out, 256t+128j+p]
        wt = np.ascontiguousarray(np.asarray(w, np.float32).T)  # [in, out]
        return np.ascontiguousarray(
            wt.reshape(2, 2, 128, C).transpose(0, 2, 1, 3)).astype(E4)

    wq8 = wprep(inputs["Wq"])
    wk8 = wprep(inputs["Wk"])
    wv8 = wprep(inputs["Wv"])
    # wp16[kc][p][out] = Wp[out, kc*128+p]
    wpT = np.asarray(inputs["Wp"], np.float32).T
    wp16 = np.ascontiguousarray(wpT.reshape(NC4, 128, C)).astype(bf)
    gamma = np.asarray(inputs["gamma"], np.float32).reshape(C, 1)
    beta = np.asarray(inputs["beta"], np.float32).reshape(C, 1)
    bp = np.asarray(inputs["bp"], np.float32).reshape(C, 1)

    ct8 = {}
    for b in range(B):
        ct8[b] = np.ascontiguousarray(
            ct[b].reshape(2, 2, 128, L).transpose(0, 2, 1, 3)).astype(E4)

    in_maps = []
    for core in range(NCORES):
        b, jq = divmod(core, 4)
        sl = slice(jq * LQ, (jq + 1) * LQ)
        pet_sl = np.ascontiguousarray(pet[b][:, sl])
        pet8 = np.ascontiguousarray(
            (pet_sl * QPRESCALE).reshape(2, 2, 128, LQ).transpose(0, 2, 1, 3)
        ).astype(E4)
        in_maps.append({
            "pet8": pet8,
            "ct8": ct8[b],
            "pet16": pet_sl.astype(bf),
            "wq8": wq8, "wk8": wk8, "wv8": wv8, "wp16": wp16,
            "gamma": gamma, "beta": beta, "bp": bp,
        })
    return in_maps


def assemble_output(results):
    out = np.empty((B, C, L), np.float32)
    for core in range(NCORES):
        b, jq = divmod(core, 4)
        out[b][:, jq * LQ:(jq + 1) * LQ] = results[core]["out"]
    return out.reshape(B, C, H, W)


_NC_CACHE = {}


def get_nc(debug=False, debug_taps=False):
    key = (debug, debug_taps)
    if key not in _NC_CACHE:
        _NC_CACHE[key] = build_nc(debug=debug, debug_taps=debug_taps)
    return _NC_CACHE[key]


def kernel(**inputs):
    nc = get_nc()
    in_maps = prep_core_inputs(inputs)
    res = bass_utils.run_bass_kernel_spmd(nc, in_maps, list(range(NCORES)))
    return assemble_output(res.results)
